# revision 2
# baseline (speedup 1.0000x reference)
"""Trainium2 Bass kernel for an 8-expert top-2 SwiGLU MoE (expert parallelism).

Structure (8 NeuronCores, one expert per core):
  - Stage 1 (gating): stream the full transposed token set xT (f32) in 16
    chunks of 512 tokens, compute logits on the PE in f32 (exact top-2
    selection), gate matrix column-permuted per core so its OWN expert is
    column 0. Top-2 via vector MAX8 per 128-token block; routing weights +
    mask via batched DVE ops; compact slot ids via matmul prefix-sums;
    per-block indirect scatters write (token_id, weight_bits) into idw_d.
  - Stage 2 (gather): indirect-gather token rows from a bf16 copy of x,
    then XBAR DMA transposes (no PE) into feature-major xgT bf16.
  - Stage 3/4 (FFN, all bf16): single pass over all C=2176 slots in 5
    slices (4x512+128). Weights live in DRAM pre-marshaled bf16 with
    per-partition-contiguous tile layout, streamed exactly ONCE on the
    scalar-engine DMA queue. pass1: h=silu(x@w1T)*(x@w3T) -> h_all bf16
    resident; pass2: yT = (h@w2T)^T in f32 to yT_d[D, C].
  - Host: y[ids] += w[:, None] * yT[:, :cnt].T per core.

Self-contained: hardcodes shapes for x[4,2048,1024], 8 experts, H=2816, top-2.
"""
import sys

sys.path.insert(0, "/opt/trn_rl_repo")

import numpy as np
from ml_dtypes import bfloat16

# ---------------------------------------------------------------- config
B, S, D = 4, 2048, 1024
T = B * S                # 8192 tokens
E = 8                    # experts == cores
H = 2816
K = 2
P = 128
NB = T // P              # 64 token blocks (token = 128*b + p)
C = 2176                 # per-expert slot capacity (observed max 2175)
NG = C // P              # 17 slot tiles
HT = H // P              # 22
DT = D // P              # 8
GATE_CHUNK = 512
NJ = T // GATE_CHUNK     # 16
BPC = GATE_CHUNK // P    # 4 blocks per gating chunk
SLICES = [(0, 512), (512, 512), (1024, 512), (1536, 512), (2048, 128)]

_cache = {}


def _build():
    import concourse.bass as bass
    import concourse.bacc as bacc
    import concourse.mybir as mybir
    import concourse.tile as tile

    f32 = mybir.dt.float32
    bf16 = mybir.dt.bfloat16
    i32 = mybir.dt.int32
    Alu = mybir.AluOpType
    Act = mybir.ActivationFunctionType

    nc = bacc.Bacc("TRN2", target_bir_lowering=False, debug=False)

    xb_d = nc.dram_tensor("xb", [T, D], bf16, kind="ExternalInput")
    xT_d = nc.dram_tensor("xT", [D, T], f32, kind="ExternalInput")
    gwP_d = nc.dram_tensor("gwP", [P, DT * E], f32, kind="ExternalInput")
    w1R_d = nc.dram_tensor("w1R", [P, HT * DT * P], bf16, kind="ExternalInput")
    w3R_d = nc.dram_tensor("w3R", [P, HT * DT * P], bf16, kind="ExternalInput")
    w2R_d = nc.dram_tensor("w2R", [P, DT * HT * P], bf16, kind="ExternalInput")
    uexc_d = nc.dram_tensor("uexc", [P, P], f32, kind="ExternalInput")
    onesc_d = nc.dram_tensor("ones_col", [P, 1], f32, kind="ExternalInput")
    onesr_d = nc.dram_tensor("ones_row", [1, P], f32, kind="ExternalInput")
    iota_d = nc.dram_tensor("iota", [P, NB], i32, kind="ExternalInput")
    ident_d = nc.dram_tensor("ident", [P, P], f32, kind="ExternalInput")

    idw_d = nc.dram_tensor("idw", [C, 2], i32, kind="ExternalOutput")
    cnt_d = nc.dram_tensor("cnt", [1, 1], f32, kind="ExternalOutput")
    yT_d = nc.dram_tensor("yT", [D, C], f32, kind="ExternalOutput")

    w1R4 = w1R_d[:].rearrange("p (t k c) -> p t k c", t=HT, k=DT)
    w3R4 = w3R_d[:].rearrange("p (t k c) -> p t k c", t=HT, k=DT)
    w2R4 = w2R_d[:].rearrange("p (d j c) -> p d j c", d=DT, j=HT)

    with tile.TileContext(nc) as tc:
        with tc.tile_pool(name="persist", bufs=1) as sp:
            # --- constants ---
            uexc = sp.tile([P, P], f32)
            nc.sync.dma_start(out=uexc[:], in_=uexc_d[:])
            onesc = sp.tile([P, 1], f32)
            nc.sync.dma_start(out=onesc[:], in_=onesc_d[:])
            onesr = sp.tile([1, P], f32)
            nc.sync.dma_start(out=onesr[:], in_=onesr_d[:])
            iota = sp.tile([P, NB], i32)
            nc.sync.dma_start(out=iota[:], in_=iota_d[:])
            ident = sp.tile([P, P], f32)
            nc.sync.dma_start(out=ident[:], in_=ident_d[:])
            gws = sp.tile([P, DT * E], f32)
            nc.sync.dma_start(out=gws[:], in_=gwP_d[:])

            # PE wait-absorber: matmul codegen allows a single sync wait, so
            # before any matmul that would need 2+ waits we make the PE observe
            # the extra semaphores through a tiny dummy matmul.
            dummy_ps = None

            def pe_touch(ap):
                n = ap.shape[-1]
                nc.tensor.matmul(dummy_ps[0:1, 0:n], lhsT=ap[:, 0:1], rhs=ap,
                                 start=True, stop=True, skip_group_check=True)

            scores = sp.tile([P, NB * E], f32)     # [p, b*E+e] logits (perm'd)
            mx_all = sp.tile([P, NB * 8], f32)     # per-block top-8 descending
            incl_all = sp.tile([1, NB], f32)

            # FFN persistent activations (bf16)
            xgT = [sp.tile([P, C], bf16, tag=f"xgT{k}", name=f"xgT{k}")
                   for k in range(DT)]
            h_all = [sp.tile([P, C], bf16, tag=f"h{ht}", name=f"h{ht}")
                     for ht in range(HT)]
            idw_sb = [sp.tile([P, 2], i32, tag=f"idw{g}", name=f"idw{g}")
                      for g in range(NG)]

            # ---------------- stage 1: gating + routing ----------------
            with tc.tile_pool(name="gpsum", bufs=2, space="PSUM") as ppg, \
                 tc.tile_pool(name="gsb", bufs=3) as sg:
                dummy_ps = ppg.tile([1, 2], f32, tag="dummy", bufs=1)
                pe_touch(gws[0:1, 0:2])
                pe_touch(ident[0:1, 0:2])
                pe_touch(uexc[0:1, 0:2])
                pe_touch(onesc[0:1, 0:1])
                pe_touch(onesr[0:1, 0:2])
                sc3 = scores[:].rearrange("p (b e) -> p b e", e=E)
                mx3 = mx_all[:].rearrange("p (b e) -> p b e", e=8)
                for j in range(NJ):
                    b0 = j * BPC
                    xt = sg.tile([P, DT, GATE_CHUNK], f32, tag="xt", bufs=3)
                    nc.sync.dma_start(
                        out=xt[:],
                        in_=xT_d[:].rearrange("(k p) t -> p k t", p=P)[:, :, j * GATE_CHUNK:(j + 1) * GATE_CHUNK])
                    ps = ppg.tile([E, GATE_CHUNK], f32, tag="ps", space="PSUM")
                    for k in range(DT):
                        nc.tensor.matmul(ps[:],
                                         lhsT=gws[:, k * E:(k + 1) * E],
                                         rhs=xt[:, k, :],
                                         start=(k == 0), stop=(k == DT - 1))
                    sc_sb = sg.tile([E, GATE_CHUNK], f32, tag="sc")
                    nc.vector.tensor_copy(out=sc_sb[:], in_=ps[:])
                    pstb = ppg.tile([P, BPC * E], f32, tag="pst", space="PSUM")
                    for i in range(BPC):
                        nc.tensor.transpose(out=pstb[:, i * E:(i + 1) * E],
                                            in_=sc_sb[:, i * P:(i + 1) * P],
                                            identity=ident[0:E, 0:E])
                    nc.vector.tensor_copy(out=scores[:, b0 * E:(b0 + BPC) * E],
                                          in_=pstb[:])
                    for i in range(BPC):
                        nc.vector.max(out=mx_all[:, (b0 + i) * 8:(b0 + i + 1) * 8],
                                      in_=scores[:, (b0 + i) * E:(b0 + i + 1) * E])

                    m1j = mx3[:, b0:b0 + BPC, 0]
                    m2j = mx3[:, b0:b0 + BPC, 1]
                    sej = sc3[:, b0:b0 + BPC, 0]     # own expert is column 0
                    dlt = sg.tile([P, BPC], f32, tag="dlt")
                    nc.vector.tensor_sub(out=dlt[:], in0=m2j, in1=m1j)
                    ed = sg.tile([P, BPC], f32, tag="ed")
                    nc.scalar.activation(out=ed[:], in_=dlt[:], func=Act.Exp)
                    den = sg.tile([P, BPC], f32, tag="den")
                    nc.vector.tensor_scalar_add(den[:], ed[:], 1.0)
                    wtop = sg.tile([P, BPC], f32, tag="wtop")
                    nc.vector.reciprocal(out=wtop[:], in_=den[:])
                    wsec = sg.tile([P, BPC], f32, tag="wsec")
                    nc.vector.tensor_scalar(out=wsec[:], in0=wtop[:], scalar1=-1.0,
                                            scalar2=1.0, op0=Alu.mult, op1=Alu.add)
                    istop = sg.tile([P, BPC], f32, tag="istop")
                    nc.vector.tensor_tensor(out=istop[:], in0=sej, in1=m1j, op=Alu.is_ge)
                    wdiff = sg.tile([P, BPC], f32, tag="wdiff")
                    nc.vector.tensor_sub(out=wdiff[:], in0=wtop[:], in1=wsec[:])
                    wE = sg.tile([P, BPC], f32, tag="wE")
                    nc.vector.tensor_tensor(out=wE[:], in0=istop[:], in1=wdiff[:], op=Alu.mult)
                    nc.vector.tensor_add(out=wE[:], in0=wE[:], in1=wsec[:])
                    maskj = sg.tile([P, BPC], f32, tag="maskj")
                    nc.vector.tensor_tensor(out=maskj[:], in0=sej, in1=m2j, op=Alu.is_ge)

                    pslot = ppg.tile([P, BPC], f32, tag="pslot", space="PSUM", bufs=1)
                    nc.tensor.matmul(pslot[:], lhsT=uexc[:], rhs=maskj[:], start=True, stop=False)
                    ptot = ppg.tile([1, BPC], f32, tag="dummy", space="PSUM", bufs=1)
                    nc.tensor.matmul(ptot[:], lhsT=onesc[:], rhs=maskj[:], start=True, stop=True)
                    tot = sg.tile([1, BPC], f32, tag="tot")
                    nc.vector.tensor_copy(out=tot[:], in_=ptot[:])
                    init = 0.0 if j == 0 else incl_all[:, b0 - 1:b0]
                    nc.vector.tensor_tensor_scan(incl_all[:, b0:b0 + BPC], tot[:], tot[:], init,
                                                 op0=Alu.add, op1=Alu.bypass)
                    excl = sg.tile([1, BPC], f32, tag="excl")
                    nc.vector.tensor_sub(out=excl[:], in0=incl_all[:, b0:b0 + BPC], in1=tot[:])
                    nc.tensor.matmul(pslot[:], lhsT=onesr[:], rhs=excl[:], start=False, stop=True)
                    slot_f = sg.tile([P, BPC], f32, tag="slot_f")
                    nc.vector.tensor_copy(out=slot_f[:], in_=pslot[:])
                    off_f = sg.tile([P, BPC], f32, tag="off_f")
                    nc.vector.tensor_scalar(out=off_f[:], in0=maskj[:], scalar1=-1e6,
                                            scalar2=1e6, op0=Alu.mult, op1=Alu.add)
                    slot_oob = sg.tile([P, BPC], f32, tag="slot_oob")
                    nc.vector.tensor_add(out=slot_oob[:], in0=slot_f[:], in1=off_f[:])
                    slot_i = sg.tile([P, BPC], i32, tag="slot_i")
                    nc.vector.tensor_copy(out=slot_i[:], in_=slot_oob[:])
                    iw = sg.tile([P, 2 * BPC], i32, tag="iw")
                    iw3 = iw[:].rearrange("p (b two) -> p b two", two=2)
                    nc.vector.tensor_copy(out=iw3[:, :, 0], in_=iota[:, b0:b0 + BPC])
                    nc.vector.tensor_copy(out=iw3[:, :, 1], in_=wE[:].bitcast(i32))
                    for i in range(BPC):
                        nc.gpsimd.indirect_dma_start(
                            out=idw_d[:],
                            out_offset=bass.IndirectOffsetOnAxis(ap=slot_i[:, i:i + 1], axis=0),
                            in_=iw[:, 2 * i:2 * i + 2], in_offset=None,
                            bounds_check=C - 1, oob_is_err=False)

                cnt_sb = sg.tile([1, 1], f32, tag="cnt")
                nc.vector.tensor_copy(out=cnt_sb[:], in_=incl_all[:, NB - 1:NB])
                nc.sync.dma_start(out=cnt_d[:], in_=cnt_sb[:])

            # ------- stage 2: gather (bf16) + XBAR transpose to xgT -------
            # ------- stage 3/4: FFN pass1 + pass2 (all bf16, one pass) ----
            with tc.tile_pool(name="f_ps", bufs=2, space="PSUM") as pp, \
                 tc.tile_pool(name="gat_sb", bufs=3) as sgt, \
                 tc.tile_pool(name="ffn_sb", bufs=3) as s1:
                dummy_ps = pp.tile([1, 2], f32, tag="dummy", bufs=1)

                for g in range(NG):
                    nc.sync.dma_start(out=idw_sb[g][:], in_=idw_d[P * g:P * (g + 1), :])
                    xg = sgt.tile([P, D], bf16, tag="xg")
                    nc.gpsimd.indirect_dma_start(
                        out=xg[:], out_offset=None, in_=xb_d[:],
                        in_offset=bass.IndirectOffsetOnAxis(ap=idw_sb[g][:, 0:1], axis=0),
                        bounds_check=T - 1, oob_is_err=False)
                    for k in range(DT):
                        nc.sync.dma_start_transpose(
                            out=xgT[k][:, g * P:(g + 1) * P],
                            in_=xg[:, P * k:P * (k + 1)])

                # FFN pass 1: h = silu(x@w1T) * (x@w3T)
                prev_silu = None
                for ht in range(HT):
                    w1b = s1.tile([P, DT, P], bf16, tag="w1b")
                    nc.scalar.dma_start(out=w1b[:], in_=w1R4[:, ht])
                    w3b = s1.tile([P, DT, P], bf16, tag="w3b")
                    nc.scalar.dma_start(out=w3b[:], in_=w3R4[:, ht])
                    for (s0, sl) in SLICES:
                        if ht == 0:
                            # absorb the XBAR-transpose sems for this slice
                            g_hi = (s0 + sl) // P - 1
                            for k in range(DT):
                                pe_touch(xgT[k][0:1, g_hi * P:g_hi * P + 2])
                        ph1 = pp.tile([P, 512], f32, tag="ph1", space="PSUM")
                        ph3 = pp.tile([P, 512], f32, tag="ph3", space="PSUM")
                        for k in range(DT):
                            nc.tensor.matmul(ph1[:, :sl], lhsT=w1b[:, k, :],
                                             rhs=xgT[k][:, s0:s0 + sl],
                                             start=(k == 0), stop=(k == DT - 1))
                        for k in range(DT):
                            nc.tensor.matmul(ph3[:, :sl], lhsT=w3b[:, k, :],
                                             rhs=xgT[k][:, s0:s0 + sl],
                                             start=(k == 0), stop=(k == DT - 1))
                        silu = s1.tile([P, 512], f32, tag="silu")
                        nc.scalar.activation(out=silu[:, :sl], in_=ph1[:, :sl], func=Act.Silu)
                        nc.vector.tensor_tensor(out=h_all[ht][:, s0:s0 + sl],
                                                in0=silu[:, :sl], in1=ph3[:, :sl],
                                                op=Alu.mult)
                        if prev_silu is not None:
                            pe_touch(prev_silu)
                        prev_silu = silu[0:1, 0:2]

                # FFN pass 2: yT = h @ w2T (feature-major, unscaled)
                for ht in range(HT):
                    pe_touch(h_all[ht][0:1, 0:2])
                for dt in range(DT):
                    w2b = s1.tile([P, HT, P], bf16, tag="w2b", bufs=2)
                    nc.scalar.dma_start(out=w2b[:], in_=w2R4[:, dt])
                    for (s0, sl) in SLICES:
                        py = pp.tile([P, 512], f32, tag="py", space="PSUM")
                        for j in range(HT):
                            nc.tensor.matmul(py[:, :sl], lhsT=w2b[:, j, :],
                                             rhs=h_all[j][:, s0:s0 + sl],
                                             start=(j == 0), stop=(j == HT - 1))
                        ysb = s1.tile([P, 512], f32, tag="ysb")
                        nc.vector.tensor_copy(out=ysb[:, :sl], in_=py[:, :sl])
                        nc.sync.dma_start(
                            out=yT_d[dt * P:(dt + 1) * P, s0:s0 + sl],
                            in_=ysb[:, :sl])

    nc.compile()
    return nc


def _marshal(x, gate_w, w1, w3, w2):
    xf = np.ascontiguousarray(x.reshape(T, D).astype(np.float32))
    xb = np.ascontiguousarray(xf.astype(bfloat16))
    xT = np.ascontiguousarray(xf.T)
    consts = {
        "uexc": np.triu(np.ones((P, P), np.float32), 1),
        "ones_col": np.ones((P, 1), np.float32),
        "ones_row": np.ones((1, P), np.float32),
        "iota": (np.arange(P)[:, None] + P * np.arange(NB)[None, :]).astype(np.int32),
        "ident": np.eye(P, dtype=np.float32),
    }
    in_maps = []
    for e in range(E):
        perm = [e] + [i for i in range(E) if i != e]
        gwT = gate_w[perm].T.astype(np.float32)                      # [D, 8]
        gwP = np.ascontiguousarray(
            gwT.reshape(DT, P, E).transpose(1, 0, 2)).reshape(P, DT * E)
        # per-partition contiguous tile layout:
        # w1R[p, t, k, c] = w1T[k*128+p, t*128+c],  w1T = w1[e].T  [D, H]
        w1T = w1[e].astype(np.float32).T
        w3T = w3[e].astype(np.float32).T
        w2T = w2[e].astype(np.float32).T                             # [H, D]
        w1R = np.ascontiguousarray(
            w1T.reshape(DT, P, HT, P).transpose(1, 2, 0, 3)).reshape(P, HT * DT * P).astype(bfloat16)
        w3R = np.ascontiguousarray(
            w3T.reshape(DT, P, HT, P).transpose(1, 2, 0, 3)).reshape(P, HT * DT * P).astype(bfloat16)
        w2R = np.ascontiguousarray(
            w2T.reshape(HT, P, DT, P).transpose(1, 2, 0, 3)).reshape(P, DT * HT * P).astype(bfloat16)
        in_maps.append({
            "xb": xb, "xT": xT, "gwP": gwP,
            "w1R": w1R, "w3R": w3R, "w2R": w2R, **consts,
        })
    return in_maps


def _numpy_fallback(x, gate_w, w1, w3, w2):
    xf = x.reshape(T, D).astype(np.float64)
    logits = xf @ gate_w.astype(np.float64).T
    p = np.exp(logits - logits.max(1, keepdims=True))
    p /= p.sum(1, keepdims=True)
    idx = np.argsort(-p, axis=1, kind="stable")[:, :K]
    vals = np.take_along_axis(p, idx, 1)
    vals /= vals.sum(1, keepdims=True)
    y = np.zeros_like(xf)
    for e in range(E):
        m = (idx == e)
        wgt = (vals * m).sum(1)
        tsel = m.any(1)
        xe = xf[tsel]
        hm = xe @ w1[e].astype(np.float64).T
        hm = hm / (1 + np.exp(-hm)) * (xe @ w3[e].astype(np.float64).T)
        y[tsel] += wgt[tsel, None] * (hm @ w2[e].astype(np.float64).T)
    return y.astype(np.float32).reshape(x.shape)


def run_spmd(x, gate_w, w1, w3, w2, trace=False):
    """Compile (cached), run on 8 cores, return results."""
    from concourse.bass_utils import run_bass_kernel_spmd
    if "nc" not in _cache:
        _cache["nc"] = _build()
    in_maps = _marshal(x, gate_w, w1, w3, w2)
    res = run_bass_kernel_spmd(_cache["nc"], in_maps, list(range(E)), trace=trace)
    return res


def kernel(x, gate_w, w1, w3, w2):
    x = np.asarray(x)
    res = run_spmd(x, gate_w, w1, w3, w2)
    y = np.zeros((T, D), np.float32)
    for e in range(E):
        r = res.results[e]
        cnt = int(round(float(r["cnt"][0, 0])))
        if cnt > C:
            return _numpy_fallback(x, gate_w, w1, w3, w2)
        ids = r["idw"][:cnt, 0]
        w = r["idw"][:cnt, 1].view(np.float32)
        rows = r["yT"][:, :cnt].T
        if len(np.unique(ids)) == cnt:
            y[ids] += w[:, None] * rows
        else:
            np.add.at(y, ids, w[:, None] * rows)
    return y.reshape(x.shape)


# revision 5
# speedup vs baseline: 1.3324x; 1.3324x over previous
"""Trainium2 Bass kernel for an 8-expert top-2 SwiGLU MoE (expert parallelism).

Structure (8 NeuronCores, one expert per core):
  - Stage 1 (gating): stream the full transposed token set (f32, host-
    marshaled chunk-contiguous so DMA descriptors are 16KB) in 16 chunks of
    512 tokens, logits on the PE in f32 (exact top-2 selection), gate matrix
    column-permuted per core so its OWN expert is column 0. Top-2 via vector
    MAX8 per 128-token block; routing weights + mask via batched DVE ops;
    compact slot ids via matmul prefix-sums; per-block indirect scatters
    write (token_id, weight_bits) alternating into TWO zero-initialized
    DRAM tensors (halves the WAW completion chain on the gpsimd queue).
  - Stage 2 (gather): merge idw_a+idw_b per slot tile (DVE int add),
    indirect-gather token rows from a bf16 copy of x, PE-transpose (bf16)
    into feature-major xgT bf16.
  - Stage 3/4 (FFN, all bf16): single pass over all C=2176 slots in 5
    slices (4x512+128). Weights pre-marshaled bf16 with per-partition-
    contiguous tile layout, streamed exactly ONCE on the scalar-engine DMA
    queue. pass1: h=silu(x@w1T)*(x@w3T) -> h_all bf16 resident; pass2:
    yT = (h@w2T)^T in f32 to yT_d[D, C].
  - Host: merge idw halves, y[ids] += w[:, None] * yT[:, :cnt].T per core.

Self-contained: hardcodes shapes for x[4,2048,1024], 8 experts, H=2816, top-2.
"""
import sys

sys.path.insert(0, "/opt/trn_rl_repo")

import numpy as np
from ml_dtypes import bfloat16

# ---------------------------------------------------------------- config
B, S, D = 4, 2048, 1024
T = B * S                # 8192 tokens
E = 8                    # experts == cores
H = 2816
K = 2
P = 128
NB = T // P              # 64 token blocks (token = 128*b + p)
C = 2176                 # per-expert slot capacity (observed max 2175)
NG = C // P              # 17 slot tiles
HT = H // P              # 22
DT = D // P              # 8
GATE_CHUNK = 512
NJ = T // GATE_CHUNK     # 16
BPC = GATE_CHUNK // P    # 4 blocks per gating chunk
SLICES = [(0, 512), (512, 512), (1024, 512), (1536, 512), (2048, 128)]

_cache = {}


def _build():
    import concourse.bass as bass
    import concourse.bacc as bacc
    import concourse.mybir as mybir
    import concourse.tile as tile

    f32 = mybir.dt.float32
    bf16 = mybir.dt.bfloat16
    i32 = mybir.dt.int32
    Alu = mybir.AluOpType
    Act = mybir.ActivationFunctionType

    nc = bacc.Bacc("TRN2", target_bir_lowering=False, debug=False)

    xb_d = nc.dram_tensor("xb", [T, D], bf16, kind="ExternalInput")
    xP_d = nc.dram_tensor("xP", [P, NJ * DT * GATE_CHUNK], f32, kind="ExternalInput")
    gwP_d = nc.dram_tensor("gwP", [P, DT * E], f32, kind="ExternalInput")
    w1R_d = nc.dram_tensor("w1R", [P, HT * DT * P], bf16, kind="ExternalInput")
    w3R_d = nc.dram_tensor("w3R", [P, HT * DT * P], bf16, kind="ExternalInput")
    w2R_d = nc.dram_tensor("w2R", [P, DT * HT * P], bf16, kind="ExternalInput")
    uexc_d = nc.dram_tensor("uexc", [P, P], f32, kind="ExternalInput")
    onesc_d = nc.dram_tensor("ones_col", [P, 1], f32, kind="ExternalInput")
    onesr_d = nc.dram_tensor("ones_row", [1, P], f32, kind="ExternalInput")
    iota_d = nc.dram_tensor("iota", [P, NB], i32, kind="ExternalInput")
    ident_d = nc.dram_tensor("ident", [P, P], f32, kind="ExternalInput")
    identb_d = nc.dram_tensor("identb", [P, P], bf16, kind="ExternalInput")

    idwa_d = nc.dram_tensor("idwa", [1, C * 2], i32, kind="ExternalOutput")
    idwb_d = nc.dram_tensor("idwb", [1, C * 2], i32, kind="ExternalOutput")
    cnt_d = nc.dram_tensor("cnt", [1, 1], f32, kind="ExternalOutput")
    yT_d = nc.dram_tensor("yT", [D, C], f32, kind="ExternalOutput")

    xP4 = xP_d[:].rearrange("p (j k t) -> p j k t", j=NJ, k=DT)
    w1R4 = w1R_d[:].rearrange("p (t k c) -> p t k c", t=HT, k=DT)
    w3R4 = w3R_d[:].rearrange("p (t k c) -> p t k c", t=HT, k=DT)
    w2R4 = w2R_d[:].rearrange("p (d j c) -> p d j c", d=DT, j=HT)
    idwa_rows = idwa_d[:].rearrange("a (c t) -> (a c) t", t=2)
    idwb_rows = idwb_d[:].rearrange("a (c t) -> (a c) t", t=2)
    idwa_pq = idwa_d[:].rearrange("a (p q) -> (a p) q", p=P)
    idwb_pq = idwb_d[:].rearrange("a (p q) -> (a p) q", p=P)

    with tile.TileContext(nc) as tc:
        with tc.tile_pool(name="persist", bufs=1) as sp:
            # --- constants ---
            uexc = sp.tile([P, P], f32)
            nc.sync.dma_start(out=uexc[:], in_=uexc_d[:])
            onesc = sp.tile([P, 1], f32)
            nc.sync.dma_start(out=onesc[:], in_=onesc_d[:])
            onesr = sp.tile([1, P], f32)
            nc.sync.dma_start(out=onesr[:], in_=onesr_d[:])
            iota = sp.tile([P, NB], i32)
            nc.sync.dma_start(out=iota[:], in_=iota_d[:])
            ident = sp.tile([P, P], f32)
            nc.sync.dma_start(out=ident[:], in_=ident_d[:])
            identb = sp.tile([P, P], bf16)
            nc.sync.dma_start(out=identb[:], in_=identb_d[:])
            gws = sp.tile([P, DT * E], f32)
            nc.sync.dma_start(out=gws[:], in_=gwP_d[:])

            # zero-init the two idw scatter targets (merge-by-add needs 0s)
            zeros_sb = sp.tile([P, C * 2 // P], i32)
            nc.vector.tensor_scalar(out=zeros_sb[:], in0=iota[:, 0:C * 2 // P],
                                    scalar1=0, scalar2=None,
                                    op0=mybir.AluOpType.mult)
            nc.sync.dma_start(out=idwa_pq, in_=zeros_sb[:])
            nc.scalar.dma_start(out=idwb_pq, in_=zeros_sb[:])

            # PE wait-absorber: matmul codegen allows a single sync wait, so
            # before any matmul that would need 2+ waits we make the PE observe
            # the extra semaphores through a tiny dummy matmul.
            dummy_ps = None

            def pe_touch(ap):
                n = ap.shape[-1]
                nc.tensor.matmul(dummy_ps[0:1, 0:n], lhsT=ap[:, 0:1], rhs=ap,
                                 start=True, stop=True, skip_group_check=True)

            scores = sp.tile([P, NB * E], f32)     # [p, b*E+e] logits (perm'd)
            mx_all = sp.tile([P, NB * 8], f32)     # per-block top-8 descending
            incl_all = sp.tile([1, NB], f32)

            # FFN persistent activations (bf16)
            xgT = [sp.tile([P, C], bf16, tag=f"xgT{k}", name=f"xgT{k}")
                   for k in range(DT)]
            h_all = [sp.tile([P, C], bf16, tag=f"h{ht}", name=f"h{ht}")
                     for ht in range(HT)]
            m_sb = [sp.tile([P, 2], i32, tag=f"m{g}", name=f"m{g}")
                    for g in range(NG)]

            # ---------------- stage 1: gating + routing ----------------
            with tc.tile_pool(name="gpsum", bufs=2, space="PSUM") as ppg, \
                 tc.tile_pool(name="gsb", bufs=3) as sg:
                dummy_ps = ppg.tile([1, 2], f32, tag="dummy", bufs=1)
                pe_touch(gws[0:1, 0:2])
                pe_touch(ident[0:1, 0:2])
                pe_touch(uexc[0:1, 0:2])
                pe_touch(onesc[0:1, 0:1])
                pe_touch(onesr[0:1, 0:2])
                sc3 = scores[:].rearrange("p (b e) -> p b e", e=E)
                mx3 = mx_all[:].rearrange("p (b e) -> p b e", e=8)
                for j in range(NJ):
                    b0 = j * BPC
                    xt = sg.tile([P, DT, GATE_CHUNK], f32, tag="xt", bufs=3)
                    nc.sync.dma_start(out=xt[:], in_=xP4[:, j])
                    ps = ppg.tile([E, GATE_CHUNK], f32, tag="ps", space="PSUM")
                    for k in range(DT):
                        nc.tensor.matmul(ps[:],
                                         lhsT=gws[:, k * E:(k + 1) * E],
                                         rhs=xt[:, k, :],
                                         start=(k == 0), stop=(k == DT - 1))
                    sc_sb = sg.tile([E, GATE_CHUNK], f32, tag="sc")
                    nc.vector.tensor_copy(out=sc_sb[:], in_=ps[:])
                    pstb = ppg.tile([P, BPC * E], f32, tag="pst", space="PSUM")
                    for i in range(BPC):
                        nc.tensor.transpose(out=pstb[:, i * E:(i + 1) * E],
                                            in_=sc_sb[:, i * P:(i + 1) * P],
                                            identity=ident[0:E, 0:E])
                    nc.vector.tensor_copy(out=scores[:, b0 * E:(b0 + BPC) * E],
                                          in_=pstb[:])
                    for i in range(BPC):
                        nc.vector.max(out=mx_all[:, (b0 + i) * 8:(b0 + i + 1) * 8],
                                      in_=scores[:, (b0 + i) * E:(b0 + i + 1) * E])

                    m1j = mx3[:, b0:b0 + BPC, 0]
                    m2j = mx3[:, b0:b0 + BPC, 1]
                    sej = sc3[:, b0:b0 + BPC, 0]     # own expert is column 0
                    dlt = sg.tile([P, BPC], f32, tag="dlt")
                    nc.vector.tensor_sub(out=dlt[:], in0=m2j, in1=m1j)
                    ed = sg.tile([P, BPC], f32, tag="ed")
                    nc.scalar.activation(out=ed[:], in_=dlt[:], func=Act.Exp)
                    den = sg.tile([P, BPC], f32, tag="den")
                    nc.vector.tensor_scalar_add(den[:], ed[:], 1.0)
                    wtop = sg.tile([P, BPC], f32, tag="wtop")
                    nc.vector.reciprocal(out=wtop[:], in_=den[:])
                    wsec = sg.tile([P, BPC], f32, tag="wsec")
                    nc.vector.tensor_scalar(out=wsec[:], in0=wtop[:], scalar1=-1.0,
                                            scalar2=1.0, op0=Alu.mult, op1=Alu.add)
                    istop = sg.tile([P, BPC], f32, tag="istop")
                    nc.vector.tensor_tensor(out=istop[:], in0=sej, in1=m1j, op=Alu.is_ge)
                    wdiff = sg.tile([P, BPC], f32, tag="wdiff")
                    nc.vector.tensor_sub(out=wdiff[:], in0=wtop[:], in1=wsec[:])
                    wE = sg.tile([P, BPC], f32, tag="wE")
                    nc.vector.tensor_tensor(out=wE[:], in0=istop[:], in1=wdiff[:], op=Alu.mult)
                    nc.vector.tensor_add(out=wE[:], in0=wE[:], in1=wsec[:])
                    maskj = sg.tile([P, BPC], f32, tag="maskj")
                    nc.vector.tensor_tensor(out=maskj[:], in0=sej, in1=m2j, op=Alu.is_ge)

                    pslot = ppg.tile([P, BPC], f32, tag="pslot", space="PSUM", bufs=1)
                    nc.tensor.matmul(pslot[:], lhsT=uexc[:], rhs=maskj[:], start=True, stop=False)
                    ptot = ppg.tile([1, BPC], f32, tag="dummy", space="PSUM", bufs=1)
                    nc.tensor.matmul(ptot[:], lhsT=onesc[:], rhs=maskj[:], start=True, stop=True)
                    tot = sg.tile([1, BPC], f32, tag="tot")
                    nc.vector.tensor_copy(out=tot[:], in_=ptot[:])
                    init = 0.0 if j == 0 else incl_all[:, b0 - 1:b0]
                    nc.vector.tensor_tensor_scan(incl_all[:, b0:b0 + BPC], tot[:], tot[:], init,
                                                 op0=Alu.add, op1=Alu.bypass)
                    excl = sg.tile([1, BPC], f32, tag="excl")
                    nc.vector.tensor_sub(out=excl[:], in0=incl_all[:, b0:b0 + BPC], in1=tot[:])
                    nc.tensor.matmul(pslot[:], lhsT=onesr[:], rhs=excl[:], start=False, stop=True)
                    slot_f = sg.tile([P, BPC], f32, tag="slot_f")
                    nc.vector.tensor_copy(out=slot_f[:], in_=pslot[:])
                    off_f = sg.tile([P, BPC], f32, tag="off_f")
                    nc.vector.tensor_scalar(out=off_f[:], in0=maskj[:], scalar1=-1e6,
                                            scalar2=1e6, op0=Alu.mult, op1=Alu.add)
                    slot_oob = sg.tile([P, BPC], f32, tag="slot_oob")
                    nc.vector.tensor_add(out=slot_oob[:], in0=slot_f[:], in1=off_f[:])
                    slot_i = sg.tile([P, BPC], i32, tag="slot_i")
                    nc.vector.tensor_copy(out=slot_i[:], in_=slot_oob[:])
                    iw = sg.tile([P, 2 * BPC], i32, tag="iw")
                    iw3 = iw[:].rearrange("p (b two) -> p b two", two=2)
                    nc.vector.tensor_copy(out=iw3[:, :, 0], in_=iota[:, b0:b0 + BPC])
                    nc.vector.tensor_copy(out=iw3[:, :, 1], in_=wE[:].bitcast(i32))
                    for i in range(BPC):
                        tgt = idwa_rows if (b0 + i) % 2 == 0 else idwb_rows
                        nc.gpsimd.indirect_dma_start(
                            out=tgt,
                            out_offset=bass.IndirectOffsetOnAxis(ap=slot_i[:, i:i + 1], axis=0),
                            in_=iw[:, 2 * i:2 * i + 2], in_offset=None,
                            bounds_check=C - 1, oob_is_err=False)

                cnt_sb = sg.tile([1, 1], f32, tag="cnt")
                nc.vector.tensor_copy(out=cnt_sb[:], in_=incl_all[:, NB - 1:NB])
                nc.sync.dma_start(out=cnt_d[:], in_=cnt_sb[:])

            # ------- stage 2: idw merge + gather + PE transpose -------
            # ------- stage 3/4: FFN pass1 + pass2 (all bf16, one pass) ----
            with tc.tile_pool(name="f_ps", bufs=2, space="PSUM") as pp, \
                 tc.tile_pool(name="gat_sb", bufs=3) as sgt, \
                 tc.tile_pool(name="ffn_sb", bufs=3) as s1:
                dummy_ps = pp.tile([1, 2], f32, tag="dummy", bufs=1)

                for g in range(NG):
                    a_sb = sgt.tile([P, 2], i32, tag="a_sb")
                    nc.sync.dma_start(out=a_sb[:], in_=idwa_rows[P * g:P * (g + 1), :])
                    b_sb = sgt.tile([P, 2], i32, tag="b_sb")
                    nc.scalar.dma_start(out=b_sb[:], in_=idwb_rows[P * g:P * (g + 1), :])
                    nc.vector.tensor_add(out=m_sb[g][:], in0=a_sb[:], in1=b_sb[:])
                    xg = sgt.tile([P, D], bf16, tag="xg")
                    nc.gpsimd.indirect_dma_start(
                        out=xg[:], out_offset=None, in_=xb_d[:],
                        in_offset=bass.IndirectOffsetOnAxis(ap=m_sb[g][:, 0:1], axis=0),
                        bounds_check=T - 1, oob_is_err=False)
                    for k in range(DT):
                        pst = pp.tile([P, P], bf16, tag="pstT", space="PSUM")
                        nc.tensor.transpose(out=pst[:], in_=xg[:, P * k:P * (k + 1)],
                                            identity=identb[:])
                        nc.vector.tensor_copy(out=xgT[k][:, g * P:(g + 1) * P],
                                              in_=pst[:])

                # FFN pass 1: h = silu(x@w1T) * (x@w3T)
                prev_silu = None
                for ht in range(HT):
                    w1b = s1.tile([P, DT, P], bf16, tag="w1b")
                    nc.scalar.dma_start(out=w1b[:], in_=w1R4[:, ht])
                    w3b = s1.tile([P, DT, P], bf16, tag="w3b")
                    nc.scalar.dma_start(out=w3b[:], in_=w3R4[:, ht])
                    for (s0, sl) in SLICES:
                        if ht == 0:
                            # absorb the gather-transpose DVE sems per slice
                            g_hi = (s0 + sl) // P - 1
                            for k in range(DT):
                                pe_touch(xgT[k][0:1, g_hi * P:g_hi * P + 2])
                        ph1 = pp.tile([P, 512], f32, tag="ph1", space="PSUM")
                        ph3 = pp.tile([P, 512], f32, tag="ph3", space="PSUM")
                        for k in range(DT):
                            nc.tensor.matmul(ph1[:, :sl], lhsT=w1b[:, k, :],
                                             rhs=xgT[k][:, s0:s0 + sl],
                                             start=(k == 0), stop=(k == DT - 1))
                        for k in range(DT):
                            nc.tensor.matmul(ph3[:, :sl], lhsT=w3b[:, k, :],
                                             rhs=xgT[k][:, s0:s0 + sl],
                                             start=(k == 0), stop=(k == DT - 1))
                        silu = s1.tile([P, 512], f32, tag="silu")
                        nc.scalar.activation(out=silu[:, :sl], in_=ph1[:, :sl], func=Act.Silu)
                        nc.vector.tensor_tensor(out=h_all[ht][:, s0:s0 + sl],
                                                in0=silu[:, :sl], in1=ph3[:, :sl],
                                                op=Alu.mult)
                        if prev_silu is not None:
                            pe_touch(prev_silu)
                        prev_silu = silu[0:1, 0:2]

                # FFN pass 2: yT = h @ w2T (feature-major, unscaled)
                for ht in range(HT):
                    pe_touch(h_all[ht][0:1, 0:2])
                for dt in range(DT):
                    w2b = s1.tile([P, HT, P], bf16, tag="w2b", bufs=2)
                    nc.scalar.dma_start(out=w2b[:], in_=w2R4[:, dt])
                    for (s0, sl) in SLICES:
                        py = pp.tile([P, 512], f32, tag="py", space="PSUM", bufs=1)
                        for j in range(HT):
                            nc.tensor.matmul(py[:, :sl], lhsT=w2b[:, j, :],
                                             rhs=h_all[j][:, s0:s0 + sl],
                                             start=(j == 0), stop=(j == HT - 1))
                        ysb = s1.tile([P, 512], f32, tag="ysb")
                        nc.vector.tensor_copy(out=ysb[:, :sl], in_=py[:, :sl])
                        nc.sync.dma_start(
                            out=yT_d[dt * P:(dt + 1) * P, s0:s0 + sl],
                            in_=ysb[:, :sl])

    nc.compile()
    return nc


def _marshal(x, gate_w, w1, w3, w2):
    xf = np.ascontiguousarray(x.reshape(T, D).astype(np.float32))
    xb = np.ascontiguousarray(xf.astype(bfloat16))
    xT = np.ascontiguousarray(xf.T)
    # chunk-contiguous gating stream: xP[p, j, k, t] = xT[k*128+p, j*512+t]
    xP = np.ascontiguousarray(
        xT.reshape(DT, P, NJ, GATE_CHUNK).transpose(1, 2, 0, 3)).reshape(P, -1)
    consts = {
        "uexc": np.triu(np.ones((P, P), np.float32), 1),
        "ones_col": np.ones((P, 1), np.float32),
        "ones_row": np.ones((1, P), np.float32),
        "iota": (np.arange(P)[:, None] + P * np.arange(NB)[None, :]).astype(np.int32),
        "ident": np.eye(P, dtype=np.float32),
        "identb": np.eye(P, dtype=np.float32).astype(bfloat16),
    }
    in_maps = []
    for e in range(E):
        perm = [e] + [i for i in range(E) if i != e]
        gwT = gate_w[perm].T.astype(np.float32)                      # [D, 8]
        gwP = np.ascontiguousarray(
            gwT.reshape(DT, P, E).transpose(1, 0, 2)).reshape(P, DT * E)
        # per-partition contiguous tile layout:
        # w1R[p, t, k, c] = w1T[k*128+p, t*128+c],  w1T = w1[e].T  [D, H]
        w1T = w1[e].astype(np.float32).T
        w3T = w3[e].astype(np.float32).T
        w2T = w2[e].astype(np.float32).T                             # [H, D]
        w1R = np.ascontiguousarray(
            w1T.reshape(DT, P, HT, P).transpose(1, 2, 0, 3)).reshape(P, HT * DT * P).astype(bfloat16)
        w3R = np.ascontiguousarray(
            w3T.reshape(DT, P, HT, P).transpose(1, 2, 0, 3)).reshape(P, HT * DT * P).astype(bfloat16)
        w2R = np.ascontiguousarray(
            w2T.reshape(HT, P, DT, P).transpose(1, 2, 0, 3)).reshape(P, DT * HT * P).astype(bfloat16)
        in_maps.append({
            "xb": xb, "xP": xP, "gwP": gwP,
            "w1R": w1R, "w3R": w3R, "w2R": w2R, **consts,
        })
    return in_maps


def _numpy_fallback(x, gate_w, w1, w3, w2):
    xf = x.reshape(T, D).astype(np.float64)
    logits = xf @ gate_w.astype(np.float64).T
    p = np.exp(logits - logits.max(1, keepdims=True))
    p /= p.sum(1, keepdims=True)
    idx = np.argsort(-p, axis=1, kind="stable")[:, :K]
    vals = np.take_along_axis(p, idx, 1)
    vals /= vals.sum(1, keepdims=True)
    y = np.zeros_like(xf)
    for e in range(E):
        m = (idx == e)
        wgt = (vals * m).sum(1)
        tsel = m.any(1)
        xe = xf[tsel]
        hm = xe @ w1[e].astype(np.float64).T
        hm = hm / (1 + np.exp(-hm)) * (xe @ w3[e].astype(np.float64).T)
        y[tsel] += wgt[tsel, None] * (hm @ w2[e].astype(np.float64).T)
    return y.astype(np.float32).reshape(x.shape)


def run_spmd(x, gate_w, w1, w3, w2, trace=False):
    """Compile (cached), run on 8 cores, return results."""
    from concourse.bass_utils import run_bass_kernel_spmd
    if "nc" not in _cache:
        _cache["nc"] = _build()
    in_maps = _marshal(x, gate_w, w1, w3, w2)
    res = run_bass_kernel_spmd(_cache["nc"], in_maps, list(range(E)), trace=trace)
    return res


def kernel(x, gate_w, w1, w3, w2):
    x = np.asarray(x)
    res = run_spmd(x, gate_w, w1, w3, w2)
    y = np.zeros((T, D), np.float32)
    for e in range(E):
        r = res.results[e]
        cnt = int(round(float(r["cnt"][0, 0])))
        if cnt > C:
            return _numpy_fallback(x, gate_w, w1, w3, w2)
        idw = (r["idwa"].reshape(C, 2).astype(np.int64)
               + r["idwb"].reshape(C, 2).astype(np.int64)).astype(np.int32)
        ids = idw[:cnt, 0]
        w = idw[:cnt, 1].view(np.float32)
        rows = r["yT"][:, :cnt].T
        if len(np.unique(ids)) == cnt:
            y[ids] += w[:, None] * rows
        else:
            np.add.at(y, ids, w[:, None] * rows)
    return y.reshape(x.shape)


# revision 9
# speedup vs baseline: 1.4194x; 1.0653x over previous
"""Trainium2 Bass kernel for an 8-expert top-2 SwiGLU MoE (expert parallelism).

Structure (8 NeuronCores, one expert per core):
  - Stage 1 (gating): stream the full transposed token set (f32, host-
    marshaled chunk-contiguous so DMA descriptors are 8KB, split across the
    two HWDGE queues) in 16 chunks of 512 tokens, logits on the PE in f32
    (exact top-2 selection), gate matrix column-permuted per core so its OWN
    expert is column 0. Top-2 via vector MAX8 per 128-token block; routing
    weights + mask via batched DVE ops; compact slot ids via matmul
    prefix-sums; per-block indirect scatters write (token_id, weight_bits)
    round-robin into FOUR zero-initialized partition-major DRAM tensors
    (issue-bound instead of completion-chained on the gpsimd queue), using
    arithmetic element offsets off = slot*34 - (slot//128)*4350 so each
    tensor is [P, NG*2] and loads back in ONE contiguous DMA.
  - Stage 2 (gather): merge the 4 idw tensors (DVE int adds), indirect-
    gather token rows from a bf16 copy of x, PE-transpose (bf16) into
    feature-major xgT bf16.
  - Stage 3/4 (FFN, all bf16): single pass over all C=2176 slots in 5
    slices (4x512+128). Weights pre-marshaled bf16 with per-partition-
    contiguous tile layout, streamed exactly ONCE on the scalar-engine DMA
    queue. pass1: h=silu(x@w1T)*(x@w3T) -> h_all bf16 resident; pass2:
    yT = (h@w2T)^T in f32 to yT_d[D, C].
  - Host: merge idw quarters, y[ids] += w[:, None] * yT[:, :cnt].T per core.

Self-contained: hardcodes shapes for x[4,2048,1024], 8 experts, H=2816, top-2.
"""
import sys

sys.path.insert(0, "/opt/trn_rl_repo")

import numpy as np
from ml_dtypes import bfloat16

# ---------------------------------------------------------------- config
B, S, D = 4, 2048, 1024
T = B * S                # 8192 tokens
E = 8                    # experts == cores
H = 2816
K = 2
P = 128
NB = T // P              # 64 token blocks (token = 128*b + p)
C = 2176                 # per-expert slot capacity (observed max 2175)
NG = C // P              # 17 slot tiles
HT = H // P              # 22
DT = D // P              # 8
GATE_CHUNK = 512
NJ = T // GATE_CHUNK     # 16
BPC = GATE_CHUNK // P    # 4 blocks per gating chunk
SLICES = [(0, 512), (512, 512), (1024, 512), (1536, 512), (2048, 128)]
NIDW = 4                 # scatter fan-out (independent WAW chains)
GW = NG * 2              # per-partition idw row: NG pairs

_cache = {}


def _build():
    import concourse.bass as bass
    import concourse.bacc as bacc
    import concourse.mybir as mybir
    import concourse.tile as tile

    f32 = mybir.dt.float32
    bf16 = mybir.dt.bfloat16
    i32 = mybir.dt.int32
    Alu = mybir.AluOpType
    Act = mybir.ActivationFunctionType

    nc = bacc.Bacc("TRN2", target_bir_lowering=False, debug=False)

    xb_d = nc.dram_tensor("xb", [T, D], bf16, kind="ExternalInput")
    xP_d = nc.dram_tensor("xP", [P, NJ * DT * GATE_CHUNK], f32, kind="ExternalInput")
    gwP_d = nc.dram_tensor("gwP", [P, DT * E], f32, kind="ExternalInput")
    w1R_d = nc.dram_tensor("w1R", [P, HT * DT * P], bf16, kind="ExternalInput")
    w3R_d = nc.dram_tensor("w3R", [P, HT * DT * P], bf16, kind="ExternalInput")
    w2R_d = nc.dram_tensor("w2R", [P, DT * HT * P], bf16, kind="ExternalInput")
    uexc_d = nc.dram_tensor("uexc", [P, P], f32, kind="ExternalInput")
    onesc_d = nc.dram_tensor("ones_col", [P, 1], f32, kind="ExternalInput")
    onesr_d = nc.dram_tensor("ones_row", [1, P], f32, kind="ExternalInput")
    iota_d = nc.dram_tensor("iota", [P, NB], i32, kind="ExternalInput")
    ident_d = nc.dram_tensor("ident", [P, P], f32, kind="ExternalInput")
    identb_d = nc.dram_tensor("identb", [P, P], bf16, kind="ExternalInput")

    idw_d = [nc.dram_tensor(f"idw{q}", [P, GW], i32, kind="ExternalOutput")
             for q in range(NIDW)]
    cnt_d = nc.dram_tensor("cnt", [1, 1], f32, kind="ExternalOutput")
    yT_d = nc.dram_tensor("yT", [D, C], f32, kind="ExternalOutput")

    xP4 = xP_d[:].rearrange("p (j k t) -> p j k t", j=NJ, k=DT)
    w1R4 = w1R_d[:].rearrange("p (t k c) -> p t k c", t=HT, k=DT)
    w3R4 = w3R_d[:].rearrange("p (t k c) -> p t k c", t=HT, k=DT)
    w2R4 = w2R_d[:].rearrange("p (d j c) -> p d j c", d=DT, j=HT)
    idw_el = [t[:].rearrange("p x -> (p x) ()") for t in idw_d]

    with tile.TileContext(nc) as tc:
        with tc.tile_pool(name="persist", bufs=1) as sp:
            # --- constants ---
            uexc = sp.tile([P, P], f32)
            nc.sync.dma_start(out=uexc[:], in_=uexc_d[:])
            onesc = sp.tile([P, 1], f32)
            nc.sync.dma_start(out=onesc[:], in_=onesc_d[:])
            onesr = sp.tile([1, P], f32)
            nc.sync.dma_start(out=onesr[:], in_=onesr_d[:])
            iota = sp.tile([P, NB], i32)
            nc.sync.dma_start(out=iota[:], in_=iota_d[:])
            ident = sp.tile([P, P], f32)
            nc.sync.dma_start(out=ident[:], in_=ident_d[:])
            identb = sp.tile([P, P], bf16)
            nc.sync.dma_start(out=identb[:], in_=identb_d[:])
            gws = sp.tile([P, DT * E], f32)
            nc.sync.dma_start(out=gws[:], in_=gwP_d[:])

            # zero-init the idw scatter targets (merge-by-add needs 0s)
            zeros_sb = sp.tile([P, GW], i32)
            nc.vector.tensor_scalar(out=zeros_sb[:], in0=iota[:, 0:GW],
                                    scalar1=0, scalar2=None, op0=Alu.mult)
            for q in range(NIDW):
                eng = nc.sync if q % 2 == 0 else nc.scalar
                eng.dma_start(out=idw_d[q][:], in_=zeros_sb[:])

            # PE wait-absorber: matmul codegen allows a single sync wait, so
            # before any matmul that would need 2+ waits we make the PE observe
            # the extra semaphores through a tiny dummy matmul.
            dummy_ps = None

            def pe_touch(ap):
                n = ap.shape[-1]
                nc.tensor.matmul(dummy_ps[0:1, 0:n], lhsT=ap[:, 0:1], rhs=ap,
                                 start=True, stop=True, skip_group_check=True)

            scores = sp.tile([P, NB * E], f32)     # [p, b*E+e] logits (perm'd)
            mx_all = sp.tile([P, NB * 8], f32)     # per-block top-8 descending
            incl_all = sp.tile([1, NB], f32)

            # FFN persistent activations (bf16)
            xgT = [sp.tile([P, C], bf16, tag=f"xgT{k}", name=f"xgT{k}")
                   for k in range(DT)]
            h_all = [sp.tile([P, C], bf16, tag=f"h{ht}", name=f"h{ht}")
                     for ht in range(HT)]
            m_all = sp.tile([P, GW], i32)

            # ---------------- stage 1: gating + routing ----------------
            with tc.tile_pool(name="gpsum", bufs=2, space="PSUM") as ppg, \
                 tc.tile_pool(name="gsb", bufs=3) as sg:
                dummy_ps = ppg.tile([1, 2], f32, tag="dummy", bufs=1)
                pe_touch(gws[0:1, 0:2])
                pe_touch(ident[0:1, 0:2])
                pe_touch(uexc[0:1, 0:2])
                pe_touch(onesc[0:1, 0:1])
                pe_touch(onesr[0:1, 0:2])
                sc3 = scores[:].rearrange("p (b e) -> p b e", e=E)
                mx3 = mx_all[:].rearrange("p (b e) -> p b e", e=8)
                for j in range(NJ):
                    b0 = j * BPC
                    xt = sg.tile([P, DT, GATE_CHUNK], f32, tag="xt", bufs=3)
                    nc.sync.dma_start(out=xt[:, 0:DT // 2], in_=xP4[:, j, 0:DT // 2])
                    nc.scalar.dma_start(out=xt[:, DT // 2:DT], in_=xP4[:, j, DT // 2:DT])
                    ps = ppg.tile([E, GATE_CHUNK], f32, tag="ps", space="PSUM")
                    for k in range(DT):
                        nc.tensor.matmul(ps[:],
                                         lhsT=gws[:, k * E:(k + 1) * E],
                                         rhs=xt[:, k, :],
                                         start=(k == 0), stop=(k == DT - 1))
                    sc_sb = sg.tile([E, GATE_CHUNK], f32, tag="sc")
                    nc.vector.tensor_copy(out=sc_sb[:], in_=ps[:])
                    pstb = ppg.tile([P, BPC * E], f32, tag="pst", space="PSUM")
                    for i in range(BPC):
                        nc.tensor.transpose(out=pstb[:, i * E:(i + 1) * E],
                                            in_=sc_sb[:, i * P:(i + 1) * P],
                                            identity=ident[0:E, 0:E])
                    nc.vector.tensor_copy(out=scores[:, b0 * E:(b0 + BPC) * E],
                                          in_=pstb[:])
                    for i in range(BPC):
                        nc.vector.max(out=mx_all[:, (b0 + i) * 8:(b0 + i + 1) * 8],
                                      in_=scores[:, (b0 + i) * E:(b0 + i + 1) * E])

                    m1j = mx3[:, b0:b0 + BPC, 0]
                    m2j = mx3[:, b0:b0 + BPC, 1]
                    sej = sc3[:, b0:b0 + BPC, 0]     # own expert is column 0
                    dlt = sg.tile([P, BPC], f32, tag="dlt")
                    nc.vector.tensor_sub(out=dlt[:], in0=m2j, in1=m1j)
                    ed = sg.tile([P, BPC], f32, tag="ed")
                    nc.scalar.activation(out=ed[:], in_=dlt[:], func=Act.Exp)
                    den = sg.tile([P, BPC], f32, tag="den")
                    nc.vector.tensor_scalar_add(den[:], ed[:], 1.0)
                    wtop = sg.tile([P, BPC], f32, tag="wtop")
                    nc.vector.reciprocal(out=wtop[:], in_=den[:])
                    wsec = sg.tile([P, BPC], f32, tag="wsec")
                    nc.vector.tensor_scalar(out=wsec[:], in0=wtop[:], scalar1=-1.0,
                                            scalar2=1.0, op0=Alu.mult, op1=Alu.add)
                    istop = sg.tile([P, BPC], f32, tag="istop")
                    nc.vector.tensor_tensor(out=istop[:], in0=sej, in1=m1j, op=Alu.is_ge)
                    wdiff = sg.tile([P, BPC], f32, tag="wdiff")
                    nc.vector.tensor_sub(out=wdiff[:], in0=wtop[:], in1=wsec[:])
                    wE = sg.tile([P, BPC], f32, tag="wE")
                    nc.vector.tensor_tensor(out=wE[:], in0=istop[:], in1=wdiff[:], op=Alu.mult)
                    nc.vector.tensor_add(out=wE[:], in0=wE[:], in1=wsec[:])
                    maskj = sg.tile([P, BPC], f32, tag="maskj")
                    nc.vector.tensor_tensor(out=maskj[:], in0=sej, in1=m2j, op=Alu.is_ge)

                    pslot = ppg.tile([P, BPC], f32, tag="pslot", space="PSUM", bufs=1)
                    nc.tensor.matmul(pslot[:], lhsT=uexc[:], rhs=maskj[:], start=True, stop=False)
                    ptot = ppg.tile([1, BPC], f32, tag="dummy", space="PSUM", bufs=1)
                    nc.tensor.matmul(ptot[:], lhsT=onesc[:], rhs=maskj[:], start=True, stop=True)
                    tot = sg.tile([1, BPC], f32, tag="tot")
                    nc.vector.tensor_copy(out=tot[:], in_=ptot[:])
                    init = 0.0 if j == 0 else incl_all[:, b0 - 1:b0]
                    nc.vector.tensor_tensor_scan(incl_all[:, b0:b0 + BPC], tot[:], tot[:], init,
                                                 op0=Alu.add, op1=Alu.bypass)
                    excl = sg.tile([1, BPC], f32, tag="excl")
                    nc.vector.tensor_sub(out=excl[:], in0=incl_all[:, b0:b0 + BPC], in1=tot[:])
                    nc.tensor.matmul(pslot[:], lhsT=onesr[:], rhs=excl[:], start=False, stop=True)
                    slot_f = sg.tile([P, BPC], f32, tag="slot_f")
                    nc.vector.tensor_copy(out=slot_f[:], in_=pslot[:])
                    off_f = sg.tile([P, BPC], f32, tag="off_f")
                    nc.vector.tensor_scalar(out=off_f[:], in0=maskj[:], scalar1=-1e6,
                                            scalar2=1e6, op0=Alu.mult, op1=Alu.add)
                    slot_oob = sg.tile([P, BPC], f32, tag="slot_oob")
                    nc.vector.tensor_add(out=slot_oob[:], in0=slot_f[:], in1=off_f[:])
                    # element offset into [P, GW]: off = s*34 - (s//128)*4350
                    # (i32 convert ROUNDS to nearest; center the fraction so
                    # round(s/128 - 0.496) == s//128 for all p in [0,128))
                    gq = sg.tile([P, BPC], f32, tag="gq")
                    nc.vector.tensor_scalar(out=gq[:], in0=slot_oob[:],
                                            scalar1=1.0 / P, scalar2=-0.49609375,
                                            op0=Alu.mult, op1=Alu.add)
                    gqi = sg.tile([P, BPC], i32, tag="gqi")
                    nc.vector.tensor_copy(out=gqi[:], in_=gq[:])
                    gqf = sg.tile([P, BPC], f32, tag="gqf")
                    nc.vector.tensor_copy(out=gqf[:], in_=gqi[:])
                    o1 = sg.tile([P, BPC], f32, tag="o1")
                    nc.vector.tensor_scalar(out=o1[:], in0=slot_oob[:],
                                            scalar1=float(GW), scalar2=None, op0=Alu.mult)
                    o2 = sg.tile([P, BPC], f32, tag="o2")
                    nc.vector.tensor_scalar(out=o2[:], in0=gqf[:],
                                            scalar1=float(P * GW - 2), scalar2=None,
                                            op0=Alu.mult)
                    o3 = sg.tile([P, BPC], f32, tag="o3")
                    nc.vector.tensor_sub(out=o3[:], in0=o1[:], in1=o2[:])
                    off_i = sg.tile([P, BPC], i32, tag="off_i")
                    nc.vector.tensor_copy(out=off_i[:], in_=o3[:])
                    iw = sg.tile([P, 2 * BPC], i32, tag="iw")
                    iw3 = iw[:].rearrange("p (b two) -> p b two", two=2)
                    nc.vector.tensor_copy(out=iw3[:, :, 0], in_=iota[:, b0:b0 + BPC])
                    nc.vector.tensor_copy(out=iw3[:, :, 1], in_=wE[:].bitcast(i32))
                    for i in range(BPC):
                        q = (b0 + i) % NIDW
                        nc.gpsimd.indirect_dma_start(
                            out=idw_el[q],
                            out_offset=bass.IndirectOffsetOnAxis(ap=off_i[:, i:i + 1], axis=0),
                            in_=iw[:, 2 * i:2 * i + 2], in_offset=None,
                            bounds_check=P * GW - 2, oob_is_err=False)

                cnt_sb = sg.tile([1, 1], f32, tag="cnt")
                nc.vector.tensor_copy(out=cnt_sb[:], in_=incl_all[:, NB - 1:NB])
                nc.sync.dma_start(out=cnt_d[:], in_=cnt_sb[:])

            # ------- stage 2: idw merge + gather + PE transpose -------
            # ------- stage 3/4: FFN pass1 + pass2 (all bf16, one pass) ----
            with tc.tile_pool(name="f_ps", bufs=2, space="PSUM") as pp, \
                 tc.tile_pool(name="gat_sb", bufs=3) as sgt, \
                 tc.tile_pool(name="ffn_sb", bufs=3) as s1:
                dummy_ps = pp.tile([1, 2], f32, tag="dummy", bufs=1)

                q_sb = []
                for q in range(NIDW):
                    t = sgt.tile([P, GW], i32, tag=f"q{q}", bufs=1)
                    eng = nc.sync if q % 2 == 0 else nc.scalar
                    eng.dma_start(out=t[:], in_=idw_d[q][:])
                    q_sb.append(t)
                m01 = sgt.tile([P, GW], i32, tag="m01", bufs=1)
                nc.vector.tensor_add(out=m01[:], in0=q_sb[0][:], in1=q_sb[1][:])
                m23 = sgt.tile([P, GW], i32, tag="m23", bufs=1)
                nc.vector.tensor_add(out=m23[:], in0=q_sb[2][:], in1=q_sb[3][:])
                nc.vector.tensor_add(out=m_all[:], in0=m01[:], in1=m23[:])

                for g in range(NG):
                    xg = sgt.tile([P, D], bf16, tag="xg")
                    nc.gpsimd.indirect_dma_start(
                        out=xg[:], out_offset=None, in_=xb_d[:],
                        in_offset=bass.IndirectOffsetOnAxis(ap=m_all[:, 2 * g:2 * g + 1], axis=0),
                        bounds_check=T - 1, oob_is_err=False)
                    for k in range(DT):
                        pst = pp.tile([P, P], bf16, tag="pstT", space="PSUM")
                        nc.tensor.transpose(out=pst[:], in_=xg[:, P * k:P * (k + 1)],
                                            identity=identb[:])
                        nc.vector.tensor_copy(out=xgT[k][:, g * P:(g + 1) * P],
                                              in_=pst[:])

                # FFN pass 1: h = silu(x@w1T) * (x@w3T)
                prev_silu = None
                for ht in range(HT):
                    w1b = s1.tile([P, DT, P], bf16, tag="w1b")
                    nc.scalar.dma_start(out=w1b[:], in_=w1R4[:, ht])
                    w3b = s1.tile([P, DT, P], bf16, tag="w3b")
                    nc.scalar.dma_start(out=w3b[:], in_=w3R4[:, ht])
                    for (s0, sl) in SLICES:
                        if ht == 0:
                            # absorb the gather-transpose DVE sems per slice
                            g_hi = (s0 + sl) // P - 1
                            for k in range(DT):
                                pe_touch(xgT[k][0:1, g_hi * P:g_hi * P + 2])
                        ph1 = pp.tile([P, 512], f32, tag="ph1", space="PSUM")
                        ph3 = pp.tile([P, 512], f32, tag="ph3", space="PSUM", bufs=1)
                        for k in range(DT):
                            nc.tensor.matmul(ph1[:, :sl], lhsT=w1b[:, k, :],
                                             rhs=xgT[k][:, s0:s0 + sl],
                                             start=(k == 0), stop=(k == DT - 1))
                        for k in range(DT):
                            nc.tensor.matmul(ph3[:, :sl], lhsT=w3b[:, k, :],
                                             rhs=xgT[k][:, s0:s0 + sl],
                                             start=(k == 0), stop=(k == DT - 1))
                        silu = s1.tile([P, 512], f32, tag="silu")
                        nc.scalar.activation(out=silu[:, :sl], in_=ph1[:, :sl], func=Act.Silu)
                        nc.vector.tensor_tensor(out=h_all[ht][:, s0:s0 + sl],
                                                in0=silu[:, :sl], in1=ph3[:, :sl],
                                                op=Alu.mult)
                        if prev_silu is not None:
                            pe_touch(prev_silu)
                        prev_silu = silu[0:1, 0:2]

                # FFN pass 2: yT = h @ w2T (feature-major, unscaled)
                for ht in range(HT):
                    pe_touch(h_all[ht][0:1, 0:2])
                for dt in range(DT):
                    w2b = s1.tile([P, HT, P], bf16, tag="w2b", bufs=2)
                    nc.scalar.dma_start(out=w2b[:], in_=w2R4[:, dt])
                    for (s0, sl) in SLICES:
                        py = pp.tile([P, 512], f32, tag="py", space="PSUM")
                        for j in range(HT):
                            nc.tensor.matmul(py[:, :sl], lhsT=w2b[:, j, :],
                                             rhs=h_all[j][:, s0:s0 + sl],
                                             start=(j == 0), stop=(j == HT - 1))
                        ysb = s1.tile([P, 512], f32, tag="ysb")
                        nc.vector.tensor_copy(out=ysb[:, :sl], in_=py[:, :sl])
                        nc.sync.dma_start(
                            out=yT_d[dt * P:(dt + 1) * P, s0:s0 + sl],
                            in_=ysb[:, :sl])

    nc.compile()
    return nc


def _marshal(x, gate_w, w1, w3, w2):
    xf = np.ascontiguousarray(x.reshape(T, D).astype(np.float32))
    xb = np.ascontiguousarray(xf.astype(bfloat16))
    xT = np.ascontiguousarray(xf.T)
    # chunk-contiguous gating stream: xP[p, j, k, t] = xT[k*128+p, j*512+t]
    xP = np.ascontiguousarray(
        xT.reshape(DT, P, NJ, GATE_CHUNK).transpose(1, 2, 0, 3)).reshape(P, -1)
    consts = {
        "uexc": np.triu(np.ones((P, P), np.float32), 1),
        "ones_col": np.ones((P, 1), np.float32),
        "ones_row": np.ones((1, P), np.float32),
        "iota": (np.arange(P)[:, None] + P * np.arange(NB)[None, :]).astype(np.int32),
        "ident": np.eye(P, dtype=np.float32),
        "identb": np.eye(P, dtype=np.float32).astype(bfloat16),
    }
    in_maps = []
    for e in range(E):
        perm = [e] + [i for i in range(E) if i != e]
        gwT = gate_w[perm].T.astype(np.float32)                      # [D, 8]
        gwP = np.ascontiguousarray(
            gwT.reshape(DT, P, E).transpose(1, 0, 2)).reshape(P, DT * E)
        # per-partition contiguous tile layout:
        # w1R[p, t, k, c] = w1T[k*128+p, t*128+c],  w1T = w1[e].T  [D, H]
        w1T = w1[e].astype(np.float32).T
        w3T = w3[e].astype(np.float32).T
        w2T = w2[e].astype(np.float32).T                             # [H, D]
        w1R = np.ascontiguousarray(
            w1T.reshape(DT, P, HT, P).transpose(1, 2, 0, 3)).reshape(P, HT * DT * P).astype(bfloat16)
        w3R = np.ascontiguousarray(
            w3T.reshape(DT, P, HT, P).transpose(1, 2, 0, 3)).reshape(P, HT * DT * P).astype(bfloat16)
        w2R = np.ascontiguousarray(
            w2T.reshape(HT, P, DT, P).transpose(1, 2, 0, 3)).reshape(P, DT * HT * P).astype(bfloat16)
        in_maps.append({
            "xb": xb, "xP": xP, "gwP": gwP,
            "w1R": w1R, "w3R": w3R, "w2R": w2R, **consts,
        })
    return in_maps


def _numpy_fallback(x, gate_w, w1, w3, w2):
    xf = x.reshape(T, D).astype(np.float64)
    logits = xf @ gate_w.astype(np.float64).T
    p = np.exp(logits - logits.max(1, keepdims=True))
    p /= p.sum(1, keepdims=True)
    idx = np.argsort(-p, axis=1, kind="stable")[:, :K]
    vals = np.take_along_axis(p, idx, 1)
    vals /= vals.sum(1, keepdims=True)
    y = np.zeros_like(xf)
    for e in range(E):
        m = (idx == e)
        wgt = (vals * m).sum(1)
        tsel = m.any(1)
        xe = xf[tsel]
        hm = xe @ w1[e].astype(np.float64).T
        hm = hm / (1 + np.exp(-hm)) * (xe @ w3[e].astype(np.float64).T)
        y[tsel] += wgt[tsel, None] * (hm @ w2[e].astype(np.float64).T)
    return y.astype(np.float32).reshape(x.shape)


def run_spmd(x, gate_w, w1, w3, w2, trace=False):
    """Compile (cached), run on 8 cores, return results."""
    from concourse.bass_utils import run_bass_kernel_spmd
    if "nc" not in _cache:
        _cache["nc"] = _build()
    in_maps = _marshal(x, gate_w, w1, w3, w2)
    res = run_bass_kernel_spmd(_cache["nc"], in_maps, list(range(E)), trace=trace)
    return res


def kernel(x, gate_w, w1, w3, w2):
    x = np.asarray(x)
    res = run_spmd(x, gate_w, w1, w3, w2)
    y = np.zeros((T, D), np.float32)
    for e in range(E):
        r = res.results[e]
        cnt = int(round(float(r["cnt"][0, 0])))
        if cnt > C:
            return _numpy_fallback(x, gate_w, w1, w3, w2)
        m = sum(r[f"idw{q}"].astype(np.int64) for q in range(NIDW)).astype(np.int32)
        idw = m.reshape(P, NG, 2).transpose(1, 0, 2).reshape(C, 2)
        ids = idw[:cnt, 0]
        w = idw[:cnt, 1].view(np.float32)
        rows = r["yT"][:, :cnt].T
        if len(np.unique(ids)) == cnt:
            y[ids] += w[:, None] * rows
        else:
            np.add.at(y, ids, w[:, None] * rows)
    return y.reshape(x.shape)


# revision 13
# speedup vs baseline: 1.4206x; 1.0009x over previous
"""Trainium2 Bass kernel for an 8-expert top-2 SwiGLU MoE (expert parallelism).

Structure (8 NeuronCores, one expert per core):
  - Stage 1 (gating): stream the full transposed token set (f32, host-
    marshaled chunk-contiguous so DMA descriptors are 8KB, split across the
    two HWDGE queues) in 16 chunks of 512 tokens, logits on the PE in f32
    (exact top-2 selection), gate matrix column-permuted per core so its OWN
    expert is column 0. Top-2 via vector MAX8 per 128-token block; routing
    weights + mask via batched DVE ops; compact slot ids via matmul
    prefix-sums; per-block indirect scatters write (token_id, weight_bits)
    round-robin into FOUR zero-initialized partition-major DRAM tensors
    (issue-bound instead of completion-chained on the gpsimd queue), using
    arithmetic element offsets off = slot*34 - (slot//128)*4350 so each
    tensor is [P, NG*2] and loads back in ONE contiguous DMA.
  - Stage 2 (gather): merge the 4 idw tensors (DVE int adds), indirect-
    gather token rows from a bf16 copy of x, PE-transpose (bf16) into
    feature-major xgT bf16.
  - Stage 3/4 (FFN, all bf16): single pass over all C=2176 slots in 5
    slices (4x512+128). Weights pre-marshaled bf16 with per-partition-
    contiguous tile layout, streamed exactly ONCE on the scalar-engine DMA
    queue. pass1: h=silu(x@w1T)*(x@w3T) -> h_all bf16 resident; pass2:
    yT = (h@w2T)^T in f32 to yT_d[D, C].
  - Host: merge idw quarters, y[ids] += w[:, None] * yT[:, :cnt].T per core.

Self-contained: hardcodes shapes for x[4,2048,1024], 8 experts, H=2816, top-2.
"""
import sys

sys.path.insert(0, "/opt/trn_rl_repo")

import numpy as np
from ml_dtypes import bfloat16

# ---------------------------------------------------------------- config
B, S, D = 4, 2048, 1024
T = B * S                # 8192 tokens
E = 8                    # experts == cores
H = 2816
K = 2
P = 128
NB = T // P              # 64 token blocks (token = 128*b + p)
C = 2176                 # per-expert slot capacity (observed max 2175)
NG = C // P              # 17 slot tiles
HT = H // P              # 22
DT = D // P              # 8
GATE_CHUNK = 1024
NJ = T // GATE_CHUNK     # 8
BPC = GATE_CHUNK // P    # 8 blocks per gating chunk
SLICES = [(0, 512), (512, 512), (1024, 512), (1536, 384), (1920, 256)]
NIDW = 4                 # scatter fan-out (independent WAW chains)
GW = NG * 2              # per-partition idw row: NG pairs

_cache = {}


def _build():
    import concourse.bass as bass
    import concourse.bacc as bacc
    import concourse.mybir as mybir
    import concourse.tile as tile

    f32 = mybir.dt.float32
    bf16 = mybir.dt.bfloat16
    i32 = mybir.dt.int32
    Alu = mybir.AluOpType
    Act = mybir.ActivationFunctionType

    nc = bacc.Bacc("TRN2", target_bir_lowering=False, debug=False)

    xb_d = nc.dram_tensor("xb", [T, D], bf16, kind="ExternalInput")
    xP_d = nc.dram_tensor("xP", [P, NJ * DT * GATE_CHUNK], f32, kind="ExternalInput")
    gwP_d = nc.dram_tensor("gwP", [P, DT * E], f32, kind="ExternalInput")
    w1R_d = nc.dram_tensor("w1R", [P, HT * DT * P], bf16, kind="ExternalInput")
    w3R_d = nc.dram_tensor("w3R", [P, HT * DT * P], bf16, kind="ExternalInput")
    w2R_d = nc.dram_tensor("w2R", [P, DT * HT * P], bf16, kind="ExternalInput")
    uexc_d = nc.dram_tensor("uexc", [P, P], f32, kind="ExternalInput")
    onesc_d = nc.dram_tensor("ones_col", [P, 1], f32, kind="ExternalInput")
    onesr_d = nc.dram_tensor("ones_row", [1, P], f32, kind="ExternalInput")
    iota_d = nc.dram_tensor("iota", [P, NB], i32, kind="ExternalInput")
    ident_d = nc.dram_tensor("ident", [P, P], f32, kind="ExternalInput")
    identb_d = nc.dram_tensor("identb", [P, P], bf16, kind="ExternalInput")

    idw_d = [nc.dram_tensor(f"idw{q}", [P, GW], i32, kind="ExternalOutput")
             for q in range(NIDW)]
    cnt_d = nc.dram_tensor("cnt", [1, 1], f32, kind="ExternalOutput")
    yT_d = nc.dram_tensor("yT", [D, C], f32, kind="ExternalOutput")

    xP4 = xP_d[:].rearrange("p (j k t) -> p j k t", j=NJ, k=DT)
    w1R4 = w1R_d[:].rearrange("p (t k c) -> p t k c", t=HT, k=DT)
    w3R4 = w3R_d[:].rearrange("p (t k c) -> p t k c", t=HT, k=DT)
    w2R4 = w2R_d[:].rearrange("p (d j c) -> p d j c", d=DT, j=HT)
    idw_el = [t[:].rearrange("p x -> (p x) ()") for t in idw_d]

    with tile.TileContext(nc) as tc:
        with tc.tile_pool(name="persist", bufs=1) as sp:
            # --- constants ---
            uexc = sp.tile([P, P], f32)
            nc.sync.dma_start(out=uexc[:], in_=uexc_d[:])
            onesc = sp.tile([P, 1], f32)
            nc.sync.dma_start(out=onesc[:], in_=onesc_d[:])
            onesr = sp.tile([1, P], f32)
            nc.sync.dma_start(out=onesr[:], in_=onesr_d[:])
            iota = sp.tile([P, NB], i32)
            nc.sync.dma_start(out=iota[:], in_=iota_d[:])
            ident = sp.tile([P, P], f32)
            nc.sync.dma_start(out=ident[:], in_=ident_d[:])
            identb = sp.tile([P, P], bf16)
            nc.sync.dma_start(out=identb[:], in_=identb_d[:])
            gws = sp.tile([P, DT * E], f32)
            nc.sync.dma_start(out=gws[:], in_=gwP_d[:])

            # zero-init the idw scatter targets (merge-by-add needs 0s)
            zeros_sb = sp.tile([P, GW], i32)
            nc.vector.tensor_scalar(out=zeros_sb[:], in0=iota[:, 0:GW],
                                    scalar1=0, scalar2=None, op0=Alu.mult)
            for q in range(NIDW):
                eng = nc.sync if q % 2 == 0 else nc.scalar
                eng.dma_start(out=idw_d[q][:], in_=zeros_sb[:])

            # PE wait-absorber: matmul codegen allows a single sync wait, so
            # before any matmul that would need 2+ waits we make the PE observe
            # the extra semaphores through a tiny dummy matmul.
            dummy_ps = None

            def pe_touch(ap):
                n = ap.shape[-1]
                nc.tensor.matmul(dummy_ps[0:1, 0:n], lhsT=ap[:, 0:1], rhs=ap,
                                 start=True, stop=True, skip_group_check=True)

            incl_all = sp.tile([1, NB], f32)

            # FFN persistent activations (bf16)
            xgT = [sp.tile([P, C], bf16, tag=f"xgT{k}", name=f"xgT{k}")
                   for k in range(DT)]
            h_all = [sp.tile([P, C], bf16, tag=f"h{ht}", name=f"h{ht}")
                     for ht in range(HT)]
            m_all = sp.tile([P, GW], i32)

            # ---------------- stage 1: gating + routing ----------------
            with tc.tile_pool(name="gpsum", bufs=2, space="PSUM") as ppg, \
                 tc.tile_pool(name="gsb", bufs=3) as sg:
                dummy_ps = ppg.tile([1, 2], f32, tag="dummy", bufs=1)
                pe_touch(gws[0:1, 0:2])
                pe_touch(ident[0:1, 0:2])
                pe_touch(uexc[0:1, 0:2])
                pe_touch(onesc[0:1, 0:1])
                pe_touch(onesr[0:1, 0:2])
                for j in range(NJ):
                    b0 = j * BPC
                    xt = sg.tile([P, DT, GATE_CHUNK], f32, tag="xt", bufs=2)
                    eng = nc.sync if j % 2 == 0 else nc.scalar
                    eng.dma_start(out=xt[:], in_=xP4[:, j])
                    ps = ppg.tile([E, GATE_CHUNK], f32, tag="ps", space="PSUM")
                    for h0 in range(0, GATE_CHUNK, 512):
                        for k in range(DT):
                            nc.tensor.matmul(ps[:, h0:h0 + 512],
                                             lhsT=gws[:, k * E:(k + 1) * E],
                                             rhs=xt[:, k, h0:h0 + 512],
                                             start=(k == 0), stop=(k == DT - 1))
                    sc_sb = sg.tile([E, GATE_CHUNK], f32, tag="sc", bufs=2)
                    nc.vector.tensor_copy(out=sc_sb[:], in_=ps[:])
                    pstb = ppg.tile([P, BPC * E], f32, tag="pst", space="PSUM")
                    for i in range(BPC):
                        nc.tensor.transpose(out=pstb[:, i * E:(i + 1) * E],
                                            in_=sc_sb[:, i * P:(i + 1) * P],
                                            identity=ident[0:E, 0:E])
                    scores = sg.tile([P, BPC * E], f32, tag="scores")
                    nc.vector.tensor_copy(out=scores[:], in_=pstb[:])
                    mx = sg.tile([P, BPC * 8], f32, tag="mx")
                    for i in range(BPC):
                        nc.vector.max(out=mx[:, i * 8:(i + 1) * 8],
                                      in_=scores[:, i * E:(i + 1) * E])

                    sc3 = scores[:].rearrange("p (b e) -> p b e", e=E)
                    mx3 = mx[:].rearrange("p (b e) -> p b e", e=8)
                    m1j = mx3[:, :, 0]
                    m2j = mx3[:, :, 1]
                    sej = sc3[:, :, 0]               # own expert is column 0
                    dlt = sg.tile([P, BPC], f32, tag="dlt")
                    nc.vector.tensor_sub(out=dlt[:], in0=m2j, in1=m1j)
                    ed = sg.tile([P, BPC], f32, tag="ed")
                    nc.scalar.activation(out=ed[:], in_=dlt[:], func=Act.Exp)
                    den = sg.tile([P, BPC], f32, tag="den")
                    nc.vector.tensor_scalar_add(den[:], ed[:], 1.0)
                    wtop = sg.tile([P, BPC], f32, tag="wtop")
                    nc.vector.reciprocal(out=wtop[:], in_=den[:])
                    wsec = sg.tile([P, BPC], f32, tag="wsec")
                    nc.vector.tensor_scalar(out=wsec[:], in0=wtop[:], scalar1=-1.0,
                                            scalar2=1.0, op0=Alu.mult, op1=Alu.add)
                    istop = sg.tile([P, BPC], f32, tag="istop")
                    nc.vector.tensor_tensor(out=istop[:], in0=sej, in1=m1j, op=Alu.is_ge)
                    wdiff = sg.tile([P, BPC], f32, tag="wdiff")
                    nc.vector.tensor_sub(out=wdiff[:], in0=wtop[:], in1=wsec[:])
                    wE = sg.tile([P, BPC], f32, tag="wE")
                    nc.vector.tensor_tensor(out=wE[:], in0=istop[:], in1=wdiff[:], op=Alu.mult)
                    nc.vector.tensor_add(out=wE[:], in0=wE[:], in1=wsec[:])
                    maskj = sg.tile([P, BPC], f32, tag="maskj")
                    nc.vector.tensor_tensor(out=maskj[:], in0=sej, in1=m2j, op=Alu.is_ge)

                    pslot = ppg.tile([P, BPC], f32, tag="pslot", space="PSUM", bufs=1)
                    nc.tensor.matmul(pslot[:], lhsT=uexc[:], rhs=maskj[:], start=True, stop=False)
                    ptot = ppg.tile([1, BPC], f32, tag="dummy", space="PSUM", bufs=1)
                    nc.tensor.matmul(ptot[:], lhsT=onesc[:], rhs=maskj[:], start=True, stop=True)
                    tot = sg.tile([1, BPC], f32, tag="tot")
                    nc.vector.tensor_copy(out=tot[:], in_=ptot[:])
                    init = 0.0 if j == 0 else incl_all[:, b0 - 1:b0]
                    nc.vector.tensor_tensor_scan(incl_all[:, b0:b0 + BPC], tot[:], tot[:], init,
                                                 op0=Alu.add, op1=Alu.bypass)
                    excl = sg.tile([1, BPC], f32, tag="excl")
                    nc.vector.tensor_sub(out=excl[:], in0=incl_all[:, b0:b0 + BPC], in1=tot[:])
                    nc.tensor.matmul(pslot[:], lhsT=onesr[:], rhs=excl[:], start=False, stop=True)
                    slot_f = sg.tile([P, BPC], f32, tag="slot_f")
                    nc.vector.tensor_copy(out=slot_f[:], in_=pslot[:])
                    off_f = sg.tile([P, BPC], f32, tag="off_f")
                    nc.vector.tensor_scalar(out=off_f[:], in0=maskj[:], scalar1=-1e6,
                                            scalar2=1e6, op0=Alu.mult, op1=Alu.add)
                    slot_oob = sg.tile([P, BPC], f32, tag="slot_oob")
                    nc.vector.tensor_add(out=slot_oob[:], in0=slot_f[:], in1=off_f[:])
                    # element offset into [P, GW]: off = s*34 - (s//128)*4350
                    # (i32 convert ROUNDS to nearest; center the fraction so
                    # round(s/128 - 0.496) == s//128 for all p in [0,128))
                    gq = sg.tile([P, BPC], f32, tag="gq")
                    nc.vector.tensor_scalar(out=gq[:], in0=slot_oob[:],
                                            scalar1=1.0 / P, scalar2=-0.49609375,
                                            op0=Alu.mult, op1=Alu.add)
                    gqi = sg.tile([P, BPC], i32, tag="gqi")
                    nc.vector.tensor_copy(out=gqi[:], in_=gq[:])
                    gqf = sg.tile([P, BPC], f32, tag="gqf")
                    nc.vector.tensor_copy(out=gqf[:], in_=gqi[:])
                    o1 = sg.tile([P, BPC], f32, tag="o1")
                    nc.vector.tensor_scalar(out=o1[:], in0=slot_oob[:],
                                            scalar1=float(GW), scalar2=None, op0=Alu.mult)
                    o2 = sg.tile([P, BPC], f32, tag="o2")
                    nc.vector.tensor_scalar(out=o2[:], in0=gqf[:],
                                            scalar1=float(P * GW - 2), scalar2=None,
                                            op0=Alu.mult)
                    o3 = sg.tile([P, BPC], f32, tag="o3")
                    nc.vector.tensor_sub(out=o3[:], in0=o1[:], in1=o2[:])
                    off_i = sg.tile([P, BPC], i32, tag="off_i")
                    nc.vector.tensor_copy(out=off_i[:], in_=o3[:])
                    iw = sg.tile([P, 2 * BPC], i32, tag="iw")
                    iw3 = iw[:].rearrange("p (b two) -> p b two", two=2)
                    nc.vector.tensor_copy(out=iw3[:, :, 0], in_=iota[:, b0:b0 + BPC])
                    nc.vector.tensor_copy(out=iw3[:, :, 1], in_=wE[:].bitcast(i32))
                    for i in range(BPC):
                        q = (b0 + i) % NIDW
                        nc.gpsimd.indirect_dma_start(
                            out=idw_el[q],
                            out_offset=bass.IndirectOffsetOnAxis(ap=off_i[:, i:i + 1], axis=0),
                            in_=iw[:, 2 * i:2 * i + 2], in_offset=None,
                            bounds_check=P * GW - 2, oob_is_err=False)

                cnt_sb = sg.tile([1, 1], f32, tag="cnt")
                nc.vector.tensor_copy(out=cnt_sb[:], in_=incl_all[:, NB - 1:NB])
                nc.sync.dma_start(out=cnt_d[:], in_=cnt_sb[:])

            # ------- stage 2: idw merge + gather + PE transpose -------
            # ------- stage 3/4: FFN pass1 + pass2 (all bf16, one pass) ----
            with tc.tile_pool(name="f_ps", bufs=2, space="PSUM") as pp, \
                 tc.tile_pool(name="gat_sb", bufs=3) as sgt, \
                 tc.tile_pool(name="ffn_sb", bufs=3) as s1:
                dummy_ps = pp.tile([1, 2], f32, tag="dummy", bufs=1)

                q_sb = []
                for q in range(NIDW):
                    t = sgt.tile([P, GW], i32, tag=f"q{q}", bufs=1)
                    eng = nc.sync if q % 2 == 0 else nc.scalar
                    eng.dma_start(out=t[:], in_=idw_d[q][:])
                    q_sb.append(t)
                m01 = sgt.tile([P, GW], i32, tag="m01", bufs=1)
                nc.vector.tensor_add(out=m01[:], in0=q_sb[0][:], in1=q_sb[1][:])
                m23 = sgt.tile([P, GW], i32, tag="m23", bufs=1)
                nc.vector.tensor_add(out=m23[:], in0=q_sb[2][:], in1=q_sb[3][:])
                nc.vector.tensor_add(out=m_all[:], in0=m01[:], in1=m23[:])

                for g in range(NG):
                    xg = sgt.tile([P, D], bf16, tag="xg")
                    nc.gpsimd.indirect_dma_start(
                        out=xg[:], out_offset=None, in_=xb_d[:],
                        in_offset=bass.IndirectOffsetOnAxis(ap=m_all[:, 2 * g:2 * g + 1], axis=0),
                        bounds_check=T - 1, oob_is_err=False)
                    for k in range(DT):
                        pst = pp.tile([P, P], bf16, tag="pstT", space="PSUM")
                        nc.tensor.transpose(out=pst[:], in_=xg[:, P * k:P * (k + 1)],
                                            identity=identb[:])
                        nc.vector.tensor_copy(out=xgT[k][:, g * P:(g + 1) * P],
                                              in_=pst[:])

                # FFN pass 1: h = silu(x@w1T) * (x@w3T)
                prev_silu = None
                for ht in range(HT):
                    w1b = s1.tile([P, DT, P], bf16, tag="w1b")
                    nc.scalar.dma_start(out=w1b[:], in_=w1R4[:, ht])
                    w3b = s1.tile([P, DT, P], bf16, tag="w3b")
                    nc.scalar.dma_start(out=w3b[:], in_=w3R4[:, ht])
                    for (s0, sl) in SLICES:
                        if ht == 0:
                            # absorb the gather-transpose DVE sems per slice
                            g_hi = (s0 + sl) // P - 1
                            for k in range(DT):
                                pe_touch(xgT[k][0:1, g_hi * P:g_hi * P + 2])
                        ph1 = pp.tile([P, 512], f32, tag="ph1", space="PSUM")
                        ph3 = pp.tile([P, 512], f32, tag="ph3", space="PSUM", bufs=1)
                        for k in range(DT):
                            nc.tensor.matmul(ph1[:, :sl], lhsT=w1b[:, k, :],
                                             rhs=xgT[k][:, s0:s0 + sl],
                                             start=(k == 0), stop=(k == DT - 1))
                        for k in range(DT):
                            nc.tensor.matmul(ph3[:, :sl], lhsT=w3b[:, k, :],
                                             rhs=xgT[k][:, s0:s0 + sl],
                                             start=(k == 0), stop=(k == DT - 1))
                        silu = s1.tile([P, 512], f32, tag="silu")
                        nc.scalar.activation(out=silu[:, :sl], in_=ph1[:, :sl], func=Act.Silu)
                        nc.vector.tensor_tensor(out=h_all[ht][:, s0:s0 + sl],
                                                in0=silu[:, :sl], in1=ph3[:, :sl],
                                                op=Alu.mult)
                        if prev_silu is not None:
                            pe_touch(prev_silu)
                        prev_silu = silu[0:1, 0:2]

                # FFN pass 2: yT = h @ w2T (feature-major, unscaled)
                for ht in range(HT):
                    pe_touch(h_all[ht][0:1, 0:2])
                for dt in range(DT):
                    w2b = s1.tile([P, HT, P], bf16, tag="w2b", bufs=2)
                    nc.scalar.dma_start(out=w2b[:], in_=w2R4[:, dt])
                    for (s0, sl) in SLICES:
                        py = pp.tile([P, 512], f32, tag="py", space="PSUM")
                        for j in range(HT):
                            nc.tensor.matmul(py[:, :sl], lhsT=w2b[:, j, :],
                                             rhs=h_all[j][:, s0:s0 + sl],
                                             start=(j == 0), stop=(j == HT - 1))
                        ysb = s1.tile([P, 512], f32, tag="ysb")
                        nc.vector.tensor_copy(out=ysb[:, :sl], in_=py[:, :sl])
                        nc.sync.dma_start(
                            out=yT_d[dt * P:(dt + 1) * P, s0:s0 + sl],
                            in_=ysb[:, :sl])

    nc.compile()
    return nc


def _marshal(x, gate_w, w1, w3, w2):
    xf = np.ascontiguousarray(x.reshape(T, D).astype(np.float32))
    xb = np.ascontiguousarray(xf.astype(bfloat16))
    xT = np.ascontiguousarray(xf.T)
    # chunk-contiguous gating stream: xP[p, j, k, t] = xT[k*128+p, j*512+t]
    xP = np.ascontiguousarray(
        xT.reshape(DT, P, NJ, GATE_CHUNK).transpose(1, 2, 0, 3)).reshape(P, -1)
    consts = {
        "uexc": np.triu(np.ones((P, P), np.float32), 1),
        "ones_col": np.ones((P, 1), np.float32),
        "ones_row": np.ones((1, P), np.float32),
        "iota": (np.arange(P)[:, None] + P * np.arange(NB)[None, :]).astype(np.int32),
        "ident": np.eye(P, dtype=np.float32),
        "identb": np.eye(P, dtype=np.float32).astype(bfloat16),
    }
    in_maps = []
    for e in range(E):
        perm = [e] + [i for i in range(E) if i != e]
        gwT = gate_w[perm].T.astype(np.float32)                      # [D, 8]
        gwP = np.ascontiguousarray(
            gwT.reshape(DT, P, E).transpose(1, 0, 2)).reshape(P, DT * E)
        # per-partition contiguous tile layout:
        # w1R[p, t, k, c] = w1T[k*128+p, t*128+c],  w1T = w1[e].T  [D, H]
        w1T = w1[e].astype(np.float32).T
        w3T = w3[e].astype(np.float32).T
        w2T = w2[e].astype(np.float32).T                             # [H, D]
        w1R = np.ascontiguousarray(
            w1T.reshape(DT, P, HT, P).transpose(1, 2, 0, 3)).reshape(P, HT * DT * P).astype(bfloat16)
        w3R = np.ascontiguousarray(
            w3T.reshape(DT, P, HT, P).transpose(1, 2, 0, 3)).reshape(P, HT * DT * P).astype(bfloat16)
        w2R = np.ascontiguousarray(
            w2T.reshape(HT, P, DT, P).transpose(1, 2, 0, 3)).reshape(P, DT * HT * P).astype(bfloat16)
        in_maps.append({
            "xb": xb, "xP": xP, "gwP": gwP,
            "w1R": w1R, "w3R": w3R, "w2R": w2R, **consts,
        })
    return in_maps


def _numpy_fallback(x, gate_w, w1, w3, w2):
    xf = x.reshape(T, D).astype(np.float64)
    logits = xf @ gate_w.astype(np.float64).T
    p = np.exp(logits - logits.max(1, keepdims=True))
    p /= p.sum(1, keepdims=True)
    idx = np.argsort(-p, axis=1, kind="stable")[:, :K]
    vals = np.take_along_axis(p, idx, 1)
    vals /= vals.sum(1, keepdims=True)
    y = np.zeros_like(xf)
    for e in range(E):
        m = (idx == e)
        wgt = (vals * m).sum(1)
        tsel = m.any(1)
        xe = xf[tsel]
        hm = xe @ w1[e].astype(np.float64).T
        hm = hm / (1 + np.exp(-hm)) * (xe @ w3[e].astype(np.float64).T)
        y[tsel] += wgt[tsel, None] * (hm @ w2[e].astype(np.float64).T)
    return y.astype(np.float32).reshape(x.shape)


def run_spmd(x, gate_w, w1, w3, w2, trace=False):
    """Compile (cached), run on 8 cores, return results."""
    from concourse.bass_utils import run_bass_kernel_spmd
    if "nc" not in _cache:
        _cache["nc"] = _build()
    in_maps = _marshal(x, gate_w, w1, w3, w2)
    res = run_bass_kernel_spmd(_cache["nc"], in_maps, list(range(E)), trace=trace)
    return res


def kernel(x, gate_w, w1, w3, w2):
    x = np.asarray(x)
    res = run_spmd(x, gate_w, w1, w3, w2)
    y = np.zeros((T, D), np.float32)
    for e in range(E):
        r = res.results[e]
        cnt = int(round(float(r["cnt"][0, 0])))
        if cnt > C:
            return _numpy_fallback(x, gate_w, w1, w3, w2)
        m = sum(r[f"idw{q}"].astype(np.int64) for q in range(NIDW)).astype(np.int32)
        idw = m.reshape(P, NG, 2).transpose(1, 0, 2).reshape(C, 2)
        ids = idw[:cnt, 0]
        w = idw[:cnt, 1].view(np.float32)
        rows = r["yT"][:, :cnt].T
        if len(np.unique(ids)) == cnt:
            y[ids] += w[:, None] * rows
        else:
            np.add.at(y, ids, w[:, None] * rows)
    return y.reshape(x.shape)


# revision 16
# speedup vs baseline: 1.4326x; 1.0084x over previous
"""Trainium2 Bass kernel for an 8-expert top-2 SwiGLU MoE (expert parallelism).

Structure (8 NeuronCores, one expert per core):
  - Stage 1 (gating): stream the full transposed token set (f32, host-
    marshaled chunk-contiguous so DMA descriptors are 8KB, split across the
    two HWDGE queues) in 16 chunks of 512 tokens, logits on the PE in f32
    (exact top-2 selection), gate matrix column-permuted per core so its OWN
    expert is column 0. Top-2 via vector MAX8 per 128-token block; routing
    weights + mask via batched DVE ops; compact slot ids via matmul
    prefix-sums; per-block indirect scatters write (token_id, weight_bits)
    round-robin into FOUR zero-initialized partition-major DRAM tensors
    (issue-bound instead of completion-chained on the gpsimd queue), using
    arithmetic element offsets off = slot*34 - (slot//128)*4350 so each
    tensor is [P, NG*2] and loads back in ONE contiguous DMA.
  - Stage 2 (gather): merge the 4 idw tensors (DVE int adds), indirect-
    gather token rows from a bf16 copy of x, PE-transpose (bf16) into
    feature-major xgT bf16.
  - Stage 3/4 (FFN, all bf16): single pass over all C=2176 slots in 5
    slices (4x512+128). Weights pre-marshaled bf16 with per-partition-
    contiguous tile layout, streamed exactly ONCE on the scalar-engine DMA
    queue. pass1: h=silu(x@w1T)*(x@w3T) -> h_all bf16 resident; pass2:
    yT = (h@w2T)^T in f32 to yT_d[D, C].
  - Host: merge idw quarters, y[ids] += w[:, None] * yT[:, :cnt].T per core.

Self-contained: hardcodes shapes for x[4,2048,1024], 8 experts, H=2816, top-2.
"""
import sys

sys.path.insert(0, "/opt/trn_rl_repo")

import numpy as np
from ml_dtypes import bfloat16

# ---------------------------------------------------------------- config
B, S, D = 4, 2048, 1024
T = B * S                # 8192 tokens
E = 8                    # experts == cores
H = 2816
K = 2
P = 128
NB = T // P              # 64 token blocks (token = 128*b + p)
C = 2176                 # per-expert slot capacity (observed max 2175)
NG = C // P              # 17 slot tiles
HT = H // P              # 22
DT = D // P              # 8
GATE_CHUNK = 512
NJ = T // GATE_CHUNK     # 16
BPC = GATE_CHUNK // P    # 4 blocks per gating chunk
SLICES = [(0, 512), (512, 512), (1024, 512), (1536, 384), (1920, 256)]
NIDW = 4                 # scatter fan-out (independent WAW chains)
GW = NG * 2              # per-partition idw row: NG pairs

_cache = {}


def _build():
    import concourse.bass as bass
    import concourse.bacc as bacc
    import concourse.mybir as mybir
    import concourse.tile as tile

    f32 = mybir.dt.float32
    bf16 = mybir.dt.bfloat16
    i32 = mybir.dt.int32
    Alu = mybir.AluOpType
    Act = mybir.ActivationFunctionType

    nc = bacc.Bacc("TRN2", target_bir_lowering=False, debug=False)

    xb_d = nc.dram_tensor("xb", [T, D], bf16, kind="ExternalInput")
    xP_d = nc.dram_tensor("xP", [P, NJ * DT * GATE_CHUNK], f32, kind="ExternalInput")
    gwP_d = nc.dram_tensor("gwP", [P, DT * E], f32, kind="ExternalInput")
    w1R_d = nc.dram_tensor("w1R", [P, HT * DT * P], bf16, kind="ExternalInput")
    w3R_d = nc.dram_tensor("w3R", [P, HT * DT * P], bf16, kind="ExternalInput")
    w2R_d = nc.dram_tensor("w2R", [P, DT * HT * P], bf16, kind="ExternalInput")
    uexc_d = nc.dram_tensor("uexc", [P, P], f32, kind="ExternalInput")
    onesc_d = nc.dram_tensor("ones_col", [P, 1], f32, kind="ExternalInput")
    onesr_d = nc.dram_tensor("ones_row", [1, P], f32, kind="ExternalInput")
    iota_d = nc.dram_tensor("iota", [P, NB], i32, kind="ExternalInput")
    ident_d = nc.dram_tensor("ident", [P, P], f32, kind="ExternalInput")
    identb_d = nc.dram_tensor("identb", [P, P], bf16, kind="ExternalInput")

    idw_d = [nc.dram_tensor(f"idw{q}", [P, GW], i32, kind="ExternalOutput")
             for q in range(NIDW)]
    cnt_d = nc.dram_tensor("cnt", [1, 1], f32, kind="ExternalOutput")
    yT_d = nc.dram_tensor("yT", [D, C], f32, kind="ExternalOutput")

    xP4 = xP_d[:].rearrange("p (j k t) -> p j k t", j=NJ, k=DT)
    w1R4 = w1R_d[:].rearrange("p (t k c) -> p t k c", t=HT, k=DT)
    w3R4 = w3R_d[:].rearrange("p (t k c) -> p t k c", t=HT, k=DT)
    w2R4 = w2R_d[:].rearrange("p (d j c) -> p d j c", d=DT, j=HT)
    idw_el = [t[:].rearrange("p x -> (p x) ()") for t in idw_d]

    with tile.TileContext(nc) as tc:
        with tc.tile_pool(name="persist", bufs=1) as sp:
            # --- constants ---
            uexc = sp.tile([P, P], f32)
            nc.sync.dma_start(out=uexc[:], in_=uexc_d[:])
            onesc = sp.tile([P, 1], f32)
            nc.sync.dma_start(out=onesc[:], in_=onesc_d[:])
            onesr = sp.tile([1, P], f32)
            nc.sync.dma_start(out=onesr[:], in_=onesr_d[:])
            iota = sp.tile([P, NB], i32)
            nc.sync.dma_start(out=iota[:], in_=iota_d[:])
            ident = sp.tile([P, P], f32)
            nc.sync.dma_start(out=ident[:], in_=ident_d[:])
            identb = sp.tile([P, P], bf16)
            nc.sync.dma_start(out=identb[:], in_=identb_d[:])
            gws = sp.tile([P, DT * E], f32)
            nc.sync.dma_start(out=gws[:], in_=gwP_d[:])

            # zero-init the idw scatter targets (merge-by-add needs 0s)
            zeros_sb = sp.tile([P, GW], i32)
            nc.vector.tensor_scalar(out=zeros_sb[:], in0=iota[:, 0:GW],
                                    scalar1=0, scalar2=None, op0=Alu.mult)
            for q in range(NIDW):
                eng = nc.sync if q % 2 == 0 else nc.scalar
                eng.dma_start(out=idw_d[q][:], in_=zeros_sb[:])

            # PE wait-absorber: matmul codegen allows a single sync wait, so
            # before any matmul that would need 2+ waits we make the PE observe
            # the extra semaphores through a tiny dummy matmul.
            dummy_ps = None

            def pe_touch(ap):
                n = ap.shape[-1]
                nc.tensor.matmul(dummy_ps[0:1, 0:n], lhsT=ap[:, 0:1], rhs=ap,
                                 start=True, stop=True, skip_group_check=True)

            incl_all = sp.tile([1, NB], f32)

            # FFN persistent activations (bf16)
            xgT = [sp.tile([P, C], bf16, tag=f"xgT{k}", name=f"xgT{k}")
                   for k in range(DT)]
            h_all = [sp.tile([P, C], bf16, tag=f"h{ht}", name=f"h{ht}")
                     for ht in range(HT)]
            m_all = sp.tile([P, GW], i32)

            # ---------------- stage 1: gating + routing ----------------
            with tc.tile_pool(name="gpsum", bufs=2, space="PSUM") as ppg, \
                 tc.tile_pool(name="gsb", bufs=3) as sg:
                dummy_ps = ppg.tile([1, 2], f32, tag="dummy", bufs=1)
                pe_touch(gws[0:1, 0:2])
                pe_touch(ident[0:1, 0:2])
                pe_touch(uexc[0:1, 0:2])
                pe_touch(onesc[0:1, 0:1])
                pe_touch(onesr[0:1, 0:2])
                for j in range(NJ):
                    b0 = j * BPC
                    xt = sg.tile([P, DT, GATE_CHUNK], f32, tag="xt", bufs=4)
                    eng = nc.sync if j % 2 == 0 else nc.scalar
                    eng.dma_start(out=xt[:], in_=xP4[:, j])
                    ps = ppg.tile([E, GATE_CHUNK], f32, tag="ps", space="PSUM")
                    for h0 in range(0, GATE_CHUNK, 512):
                        for k in range(DT):
                            nc.tensor.matmul(ps[:, h0:h0 + 512],
                                             lhsT=gws[:, k * E:(k + 1) * E],
                                             rhs=xt[:, k, h0:h0 + 512],
                                             start=(k == 0), stop=(k == DT - 1))
                    sc_sb = sg.tile([E, GATE_CHUNK], f32, tag="sc", bufs=2)
                    nc.vector.tensor_copy(out=sc_sb[:], in_=ps[:])
                    pstb = ppg.tile([P, BPC * E], f32, tag="pst", space="PSUM")
                    for i in range(BPC):
                        nc.tensor.transpose(out=pstb[:, i * E:(i + 1) * E],
                                            in_=sc_sb[:, i * P:(i + 1) * P],
                                            identity=ident[0:E, 0:E])
                    scores = sg.tile([P, BPC * E], f32, tag="scores")
                    nc.vector.tensor_copy(out=scores[:], in_=pstb[:])
                    mx = sg.tile([P, BPC * 8], f32, tag="mx")
                    for i in range(BPC):
                        nc.vector.max(out=mx[:, i * 8:(i + 1) * 8],
                                      in_=scores[:, i * E:(i + 1) * E])

                    sc3 = scores[:].rearrange("p (b e) -> p b e", e=E)
                    mx3 = mx[:].rearrange("p (b e) -> p b e", e=8)
                    m1j = mx3[:, :, 0]
                    m2j = mx3[:, :, 1]
                    sej = sc3[:, :, 0]               # own expert is column 0
                    dlt = sg.tile([P, BPC], f32, tag="dlt")
                    nc.vector.tensor_sub(out=dlt[:], in0=m2j, in1=m1j)
                    ed = sg.tile([P, BPC], f32, tag="ed")
                    nc.scalar.activation(out=ed[:], in_=dlt[:], func=Act.Exp)
                    den = sg.tile([P, BPC], f32, tag="den")
                    nc.vector.tensor_scalar_add(den[:], ed[:], 1.0)
                    wtop = sg.tile([P, BPC], f32, tag="wtop")
                    nc.vector.reciprocal(out=wtop[:], in_=den[:])
                    wsec = sg.tile([P, BPC], f32, tag="wsec")
                    nc.vector.tensor_scalar(out=wsec[:], in0=wtop[:], scalar1=-1.0,
                                            scalar2=1.0, op0=Alu.mult, op1=Alu.add)
                    istop = sg.tile([P, BPC], f32, tag="istop")
                    nc.vector.tensor_tensor(out=istop[:], in0=sej, in1=m1j, op=Alu.is_ge)
                    wdiff = sg.tile([P, BPC], f32, tag="wdiff")
                    nc.vector.tensor_sub(out=wdiff[:], in0=wtop[:], in1=wsec[:])
                    wE = sg.tile([P, BPC], f32, tag="wE")
                    nc.vector.tensor_tensor(out=wE[:], in0=istop[:], in1=wdiff[:], op=Alu.mult)
                    nc.vector.tensor_add(out=wE[:], in0=wE[:], in1=wsec[:])
                    maskj = sg.tile([P, BPC], f32, tag="maskj")
                    nc.vector.tensor_tensor(out=maskj[:], in0=sej, in1=m2j, op=Alu.is_ge)

                    pslot = ppg.tile([P, BPC], f32, tag="pslot", space="PSUM", bufs=1)
                    nc.tensor.matmul(pslot[:], lhsT=uexc[:], rhs=maskj[:], start=True, stop=False)
                    ptot = ppg.tile([1, BPC], f32, tag="dummy", space="PSUM", bufs=1)
                    nc.tensor.matmul(ptot[:], lhsT=onesc[:], rhs=maskj[:], start=True, stop=True)
                    tot = sg.tile([1, BPC], f32, tag="tot")
                    nc.vector.tensor_copy(out=tot[:], in_=ptot[:])
                    init = 0.0 if j == 0 else incl_all[:, b0 - 1:b0]
                    nc.vector.tensor_tensor_scan(incl_all[:, b0:b0 + BPC], tot[:], tot[:], init,
                                                 op0=Alu.add, op1=Alu.bypass)
                    excl = sg.tile([1, BPC], f32, tag="excl")
                    nc.vector.tensor_sub(out=excl[:], in0=incl_all[:, b0:b0 + BPC], in1=tot[:])
                    nc.tensor.matmul(pslot[:], lhsT=onesr[:], rhs=excl[:], start=False, stop=True)
                    slot_f = sg.tile([P, BPC], f32, tag="slot_f")
                    nc.vector.tensor_copy(out=slot_f[:], in_=pslot[:])
                    off_f = sg.tile([P, BPC], f32, tag="off_f")
                    nc.vector.tensor_scalar(out=off_f[:], in0=maskj[:], scalar1=-1e6,
                                            scalar2=1e6, op0=Alu.mult, op1=Alu.add)
                    slot_oob = sg.tile([P, BPC], f32, tag="slot_oob")
                    nc.vector.tensor_add(out=slot_oob[:], in0=slot_f[:], in1=off_f[:])
                    # element offset into [P, GW]: off = s*34 - (s//128)*4350
                    # (i32 convert ROUNDS to nearest; center the fraction so
                    # round(s/128 - 0.496) == s//128 for all p in [0,128))
                    gq = sg.tile([P, BPC], f32, tag="gq")
                    nc.vector.tensor_scalar(out=gq[:], in0=slot_oob[:],
                                            scalar1=1.0 / P, scalar2=-0.49609375,
                                            op0=Alu.mult, op1=Alu.add)
                    gqi = sg.tile([P, BPC], i32, tag="gqi")
                    nc.vector.tensor_copy(out=gqi[:], in_=gq[:])
                    gqf = sg.tile([P, BPC], f32, tag="gqf")
                    nc.vector.tensor_copy(out=gqf[:], in_=gqi[:])
                    o1 = sg.tile([P, BPC], f32, tag="o1")
                    nc.vector.tensor_scalar(out=o1[:], in0=slot_oob[:],
                                            scalar1=float(GW), scalar2=None, op0=Alu.mult)
                    o2 = sg.tile([P, BPC], f32, tag="o2")
                    nc.vector.tensor_scalar(out=o2[:], in0=gqf[:],
                                            scalar1=float(P * GW - 2), scalar2=None,
                                            op0=Alu.mult)
                    o3 = sg.tile([P, BPC], f32, tag="o3")
                    nc.vector.tensor_sub(out=o3[:], in0=o1[:], in1=o2[:])
                    off_i = sg.tile([P, BPC], i32, tag="off_i")
                    nc.vector.tensor_copy(out=off_i[:], in_=o3[:])
                    iw = sg.tile([P, 2 * BPC], i32, tag="iw")
                    iw3 = iw[:].rearrange("p (b two) -> p b two", two=2)
                    nc.vector.tensor_copy(out=iw3[:, :, 0], in_=iota[:, b0:b0 + BPC])
                    nc.vector.tensor_copy(out=iw3[:, :, 1], in_=wE[:].bitcast(i32))
                    for i in range(BPC):
                        q = (b0 + i) % NIDW
                        nc.gpsimd.indirect_dma_start(
                            out=idw_el[q],
                            out_offset=bass.IndirectOffsetOnAxis(ap=off_i[:, i:i + 1], axis=0),
                            in_=iw[:, 2 * i:2 * i + 2], in_offset=None,
                            bounds_check=P * GW - 2, oob_is_err=False)

                cnt_sb = sg.tile([1, 1], f32, tag="cnt")
                nc.vector.tensor_copy(out=cnt_sb[:], in_=incl_all[:, NB - 1:NB])
                nc.sync.dma_start(out=cnt_d[:], in_=cnt_sb[:])

            # ------- stage 2: idw merge + gather + PE transpose -------
            # ------- stage 3/4: FFN pass1 + pass2 (all bf16, one pass) ----
            with tc.tile_pool(name="f_ps", bufs=2, space="PSUM") as pp, \
                 tc.tile_pool(name="gat_sb", bufs=3) as sgt, \
                 tc.tile_pool(name="ffn_sb", bufs=3) as s1:
                dummy_ps = pp.tile([1, 2], f32, tag="dummy", bufs=1)

                q_sb = []
                for q in range(NIDW):
                    t = sgt.tile([P, GW], i32, tag=f"q{q}", bufs=1)
                    eng = nc.sync if q % 2 == 0 else nc.scalar
                    eng.dma_start(out=t[:], in_=idw_d[q][:])
                    q_sb.append(t)
                m01 = sgt.tile([P, GW], i32, tag="m01", bufs=1)
                nc.vector.tensor_add(out=m01[:], in0=q_sb[0][:], in1=q_sb[1][:])
                m23 = sgt.tile([P, GW], i32, tag="m23", bufs=1)
                nc.vector.tensor_add(out=m23[:], in0=q_sb[2][:], in1=q_sb[3][:])
                nc.vector.tensor_add(out=m_all[:], in0=m01[:], in1=m23[:])

                def emit_gathers(g0, g1):
                    # gather + PE-transpose slot tiles [g0, g1)
                    for g in range(g0, g1):
                        xg = sgt.tile([P, D], bf16, tag="xg")
                        nc.gpsimd.indirect_dma_start(
                            out=xg[:], out_offset=None, in_=xb_d[:],
                            in_offset=bass.IndirectOffsetOnAxis(ap=m_all[:, 2 * g:2 * g + 1], axis=0),
                            bounds_check=T - 1, oob_is_err=False)
                        for k in range(DT):
                            pst = pp.tile([P, P], bf16, tag="pstT", space="PSUM")
                            nc.tensor.transpose(out=pst[:], in_=xg[:, P * k:P * (k + 1)],
                                                identity=identb[:])
                            nc.vector.tensor_copy(out=xgT[k][:, g * P:(g + 1) * P],
                                                  in_=pst[:])

                # FFN pass 1: h = silu(x@w1T) * (x@w3T)
                # gathers are interleaved into ht=0's slice loop so pass1
                # matmuls start as soon as the first slice's tiles land
                g_done = 0
                prev_silu = None
                for ht in range(HT):
                    w1b = s1.tile([P, DT, P], bf16, tag="w1b")
                    nc.scalar.dma_start(out=w1b[:], in_=w1R4[:, ht])
                    w3b = s1.tile([P, DT, P], bf16, tag="w3b")
                    nc.scalar.dma_start(out=w3b[:], in_=w3R4[:, ht])
                    for (s0, sl) in SLICES:
                        if ht == 0:
                            g_need = (s0 + sl + P - 1) // P
                            emit_gathers(g_done, g_need)
                            g_done = max(g_done, g_need)
                            # absorb the gather-transpose DVE sems per slice
                            g_hi = (s0 + sl) // P - 1
                            for k in range(DT):
                                pe_touch(xgT[k][0:1, g_hi * P:g_hi * P + 2])
                        ph1 = pp.tile([P, 512], f32, tag="ph1", space="PSUM")
                        ph3 = pp.tile([P, 512], f32, tag="ph3", space="PSUM", bufs=1)
                        for k in range(DT):
                            nc.tensor.matmul(ph1[:, :sl], lhsT=w1b[:, k, :],
                                             rhs=xgT[k][:, s0:s0 + sl],
                                             start=(k == 0), stop=(k == DT - 1))
                        for k in range(DT):
                            nc.tensor.matmul(ph3[:, :sl], lhsT=w3b[:, k, :],
                                             rhs=xgT[k][:, s0:s0 + sl],
                                             start=(k == 0), stop=(k == DT - 1))
                        silu = s1.tile([P, 512], f32, tag="silu")
                        nc.scalar.activation(out=silu[:, :sl], in_=ph1[:, :sl], func=Act.Silu)
                        nc.vector.tensor_tensor(out=h_all[ht][:, s0:s0 + sl],
                                                in0=silu[:, :sl], in1=ph3[:, :sl],
                                                op=Alu.mult)
                        if prev_silu is not None:
                            pe_touch(prev_silu)
                        prev_silu = silu[0:1, 0:2]

                # FFN pass 2: yT = h @ w2T (feature-major, unscaled)
                for ht in range(HT):
                    pe_touch(h_all[ht][0:1, 0:2])
                for dt in range(DT):
                    w2b = s1.tile([P, HT, P], bf16, tag="w2b", bufs=2)
                    nc.scalar.dma_start(out=w2b[:], in_=w2R4[:, dt])
                    for (s0, sl) in SLICES:
                        py = pp.tile([P, 512], f32, tag="py", space="PSUM")
                        for j in range(HT):
                            nc.tensor.matmul(py[:, :sl], lhsT=w2b[:, j, :],
                                             rhs=h_all[j][:, s0:s0 + sl],
                                             start=(j == 0), stop=(j == HT - 1))
                        ysb = s1.tile([P, 512], f32, tag="ysb")
                        nc.vector.tensor_copy(out=ysb[:, :sl], in_=py[:, :sl])
                        nc.sync.dma_start(
                            out=yT_d[dt * P:(dt + 1) * P, s0:s0 + sl],
                            in_=ysb[:, :sl])

    nc.compile()
    return nc


def _marshal(x, gate_w, w1, w3, w2):
    xf = np.ascontiguousarray(x.reshape(T, D).astype(np.float32))
    xb = np.ascontiguousarray(xf.astype(bfloat16))
    xT = np.ascontiguousarray(xf.T)
    # chunk-contiguous gating stream: xP[p, j, k, t] = xT[k*128+p, j*512+t]
    xP = np.ascontiguousarray(
        xT.reshape(DT, P, NJ, GATE_CHUNK).transpose(1, 2, 0, 3)).reshape(P, -1)
    consts = {
        "uexc": np.triu(np.ones((P, P), np.float32), 1),
        "ones_col": np.ones((P, 1), np.float32),
        "ones_row": np.ones((1, P), np.float32),
        "iota": (np.arange(P)[:, None] + P * np.arange(NB)[None, :]).astype(np.int32),
        "ident": np.eye(P, dtype=np.float32),
        "identb": np.eye(P, dtype=np.float32).astype(bfloat16),
    }
    in_maps = []
    for e in range(E):
        perm = [e] + [i for i in range(E) if i != e]
        gwT = gate_w[perm].T.astype(np.float32)                      # [D, 8]
        gwP = np.ascontiguousarray(
            gwT.reshape(DT, P, E).transpose(1, 0, 2)).reshape(P, DT * E)
        # per-partition contiguous tile layout:
        # w1R[p, t, k, c] = w1T[k*128+p, t*128+c],  w1T = w1[e].T  [D, H]
        w1T = w1[e].astype(np.float32).T
        w3T = w3[e].astype(np.float32).T
        w2T = w2[e].astype(np.float32).T                             # [H, D]
        w1R = np.ascontiguousarray(
            w1T.reshape(DT, P, HT, P).transpose(1, 2, 0, 3)).reshape(P, HT * DT * P).astype(bfloat16)
        w3R = np.ascontiguousarray(
            w3T.reshape(DT, P, HT, P).transpose(1, 2, 0, 3)).reshape(P, HT * DT * P).astype(bfloat16)
        w2R = np.ascontiguousarray(
            w2T.reshape(HT, P, DT, P).transpose(1, 2, 0, 3)).reshape(P, DT * HT * P).astype(bfloat16)
        in_maps.append({
            "xb": xb, "xP": xP, "gwP": gwP,
            "w1R": w1R, "w3R": w3R, "w2R": w2R, **consts,
        })
    return in_maps


def _numpy_fallback(x, gate_w, w1, w3, w2):
    xf = x.reshape(T, D).astype(np.float64)
    logits = xf @ gate_w.astype(np.float64).T
    p = np.exp(logits - logits.max(1, keepdims=True))
    p /= p.sum(1, keepdims=True)
    idx = np.argsort(-p, axis=1, kind="stable")[:, :K]
    vals = np.take_along_axis(p, idx, 1)
    vals /= vals.sum(1, keepdims=True)
    y = np.zeros_like(xf)
    for e in range(E):
        m = (idx == e)
        wgt = (vals * m).sum(1)
        tsel = m.any(1)
        xe = xf[tsel]
        hm = xe @ w1[e].astype(np.float64).T
        hm = hm / (1 + np.exp(-hm)) * (xe @ w3[e].astype(np.float64).T)
        y[tsel] += wgt[tsel, None] * (hm @ w2[e].astype(np.float64).T)
    return y.astype(np.float32).reshape(x.shape)


def run_spmd(x, gate_w, w1, w3, w2, trace=False):
    """Compile (cached), run on 8 cores, return results."""
    from concourse.bass_utils import run_bass_kernel_spmd
    if "nc" not in _cache:
        _cache["nc"] = _build()
    in_maps = _marshal(x, gate_w, w1, w3, w2)
    res = run_bass_kernel_spmd(_cache["nc"], in_maps, list(range(E)), trace=trace)
    return res


def kernel(x, gate_w, w1, w3, w2):
    x = np.asarray(x)
    res = run_spmd(x, gate_w, w1, w3, w2)
    y = np.zeros((T, D), np.float32)
    for e in range(E):
        r = res.results[e]
        cnt = int(round(float(r["cnt"][0, 0])))
        if cnt > C:
            return _numpy_fallback(x, gate_w, w1, w3, w2)
        m = sum(r[f"idw{q}"].astype(np.int64) for q in range(NIDW)).astype(np.int32)
        idw = m.reshape(P, NG, 2).transpose(1, 0, 2).reshape(C, 2)
        ids = idw[:cnt, 0]
        w = idw[:cnt, 1].view(np.float32)
        rows = r["yT"][:, :cnt].T
        if len(np.unique(ids)) == cnt:
            y[ids] += w[:, None] * rows
        else:
            np.add.at(y, ids, w[:, None] * rows)
    return y.reshape(x.shape)


# revision 17
# speedup vs baseline: 1.4676x; 1.0244x over previous
"""Trainium2 Bass kernel for an 8-expert top-2 SwiGLU MoE (expert parallelism).

Structure (8 NeuronCores, one expert per core):
  - Stage 1 (gating): stream the full transposed token set (f32, host-
    marshaled chunk-contiguous so DMA descriptors are 8KB, split across the
    two HWDGE queues) in 16 chunks of 512 tokens, logits on the PE in f32
    (exact top-2 selection), gate matrix column-permuted per core so its OWN
    expert is column 0. Top-2 via vector MAX8 per 128-token block; routing
    weights + mask via batched DVE ops; compact slot ids via matmul
    prefix-sums; per-block indirect scatters write (token_id, weight_bits)
    round-robin into FOUR zero-initialized partition-major DRAM tensors
    (issue-bound instead of completion-chained on the gpsimd queue), using
    arithmetic element offsets off = slot*34 - (slot//128)*4350 so each
    tensor is [P, NG*2] and loads back in ONE contiguous DMA.
  - Stage 2 (gather): merge the 4 idw tensors (DVE int adds), indirect-
    gather token rows from a bf16 copy of x, PE-transpose (bf16) into
    feature-major xgT bf16.
  - Stage 3/4 (FFN, all bf16): single pass over all C=2176 slots in 5
    slices (4x512+128). Weights pre-marshaled bf16 with per-partition-
    contiguous tile layout, streamed exactly ONCE on the scalar-engine DMA
    queue. pass1: h=silu(x@w1T)*(x@w3T) -> h_all bf16 resident; pass2:
    yT = (h@w2T)^T in f32 to yT_d[D, C].
  - Host: merge idw quarters, y[ids] += w[:, None] * yT[:, :cnt].T per core.

Self-contained: hardcodes shapes for x[4,2048,1024], 8 experts, H=2816, top-2.
"""
import sys

sys.path.insert(0, "/opt/trn_rl_repo")

import numpy as np
from ml_dtypes import bfloat16

# ---------------------------------------------------------------- config
B, S, D = 4, 2048, 1024
T = B * S                # 8192 tokens
E = 8                    # experts == cores
H = 2816
K = 2
P = 128
NB = T // P              # 64 token blocks (token = 128*b + p)
C = 2176                 # per-expert slot capacity (observed max 2175)
NG = C // P              # 17 slot tiles
HT = H // P              # 22
DT = D // P              # 8
GATE_CHUNK = 512
NJ = T // GATE_CHUNK     # 16
BPC = GATE_CHUNK // P    # 4 blocks per gating chunk
SLICES = [(0, 512), (512, 512), (1024, 512), (1536, 384), (1920, 256)]
NIDW = 4                 # scatter fan-out (independent WAW chains)
GW = NG * 2              # per-partition idw row: NG pairs

_cache = {}


def _build():
    import concourse.bass as bass
    import concourse.bacc as bacc
    import concourse.mybir as mybir
    import concourse.tile as tile

    f32 = mybir.dt.float32
    bf16 = mybir.dt.bfloat16
    i32 = mybir.dt.int32
    Alu = mybir.AluOpType
    Act = mybir.ActivationFunctionType

    nc = bacc.Bacc("TRN2", target_bir_lowering=False, debug=False)

    xb_d = nc.dram_tensor("xb", [T, D], bf16, kind="ExternalInput")
    xP_d = nc.dram_tensor("xP", [P, NJ * DT * GATE_CHUNK], f32, kind="ExternalInput")
    gwP_d = nc.dram_tensor("gwP", [P, DT * E], f32, kind="ExternalInput")
    w1R_d = nc.dram_tensor("w1R", [P, HT * DT * P], bf16, kind="ExternalInput")
    w3R_d = nc.dram_tensor("w3R", [P, HT * DT * P], bf16, kind="ExternalInput")
    w2R_d = nc.dram_tensor("w2R", [P, DT * HT * P], bf16, kind="ExternalInput")
    uexc_d = nc.dram_tensor("uexc", [P, P], f32, kind="ExternalInput")
    onesc_d = nc.dram_tensor("ones_col", [P, 1], f32, kind="ExternalInput")
    onesr_d = nc.dram_tensor("ones_row", [1, P], f32, kind="ExternalInput")
    iota_d = nc.dram_tensor("iota", [P, NB], i32, kind="ExternalInput")
    ident_d = nc.dram_tensor("ident", [P, P], f32, kind="ExternalInput")
    identb_d = nc.dram_tensor("identb", [P, P], bf16, kind="ExternalInput")

    idw_d = [nc.dram_tensor(f"idw{q}", [P, GW], i32, kind="ExternalOutput")
             for q in range(NIDW)]
    cnt_d = nc.dram_tensor("cnt", [1, 1], f32, kind="ExternalOutput")
    yT_d = nc.dram_tensor("yT", [D, C], f32, kind="ExternalOutput")

    xP4 = xP_d[:].rearrange("p (j k t) -> p j k t", j=NJ, k=DT)
    w1R4 = w1R_d[:].rearrange("p (t k c) -> p t k c", t=HT, k=DT)
    w3R4 = w3R_d[:].rearrange("p (t k c) -> p t k c", t=HT, k=DT)
    w2R4 = w2R_d[:].rearrange("p (d j c) -> p d j c", d=DT, j=HT)
    idw_el = [t[:].rearrange("p x -> (p x) ()") for t in idw_d]

    with tile.TileContext(nc) as tc:
        with tc.tile_pool(name="persist", bufs=1) as sp:
            # --- constants ---
            uexc = sp.tile([P, P], f32)
            nc.sync.dma_start(out=uexc[:], in_=uexc_d[:])
            onesc = sp.tile([P, 1], f32)
            nc.sync.dma_start(out=onesc[:], in_=onesc_d[:])
            onesr = sp.tile([1, P], f32)
            nc.sync.dma_start(out=onesr[:], in_=onesr_d[:])
            iota = sp.tile([P, NB], i32)
            nc.sync.dma_start(out=iota[:], in_=iota_d[:])
            ident = sp.tile([P, P], f32)
            nc.sync.dma_start(out=ident[:], in_=ident_d[:])
            identb = sp.tile([P, P], bf16)
            nc.sync.dma_start(out=identb[:], in_=identb_d[:])
            gws = sp.tile([P, DT * E], f32)
            nc.sync.dma_start(out=gws[:], in_=gwP_d[:])

            # zero-init the idw scatter targets (merge-by-add needs 0s)
            zeros_sb = sp.tile([P, GW], i32)
            nc.vector.tensor_scalar(out=zeros_sb[:], in0=iota[:, 0:GW],
                                    scalar1=0, scalar2=None, op0=Alu.mult)
            for q in range(NIDW):
                eng = nc.sync if q % 2 == 0 else nc.scalar
                eng.dma_start(out=idw_d[q][:], in_=zeros_sb[:])

            # PE wait-absorber: matmul codegen allows a single sync wait, so
            # before any matmul that would need 2+ waits we make the PE observe
            # the extra semaphores through a tiny dummy matmul.
            dummy_ps = None

            def pe_touch(ap):
                n = ap.shape[-1]
                nc.tensor.matmul(dummy_ps[0:1, 0:n], lhsT=ap[:, 0:1], rhs=ap,
                                 start=True, stop=True, skip_group_check=True)

            incl_all = sp.tile([1, NB], f32)

            # FFN persistent activations (bf16)
            xgT = [sp.tile([P, C], bf16, tag=f"xgT{k}", name=f"xgT{k}")
                   for k in range(DT)]
            h_all = [sp.tile([P, C], bf16, tag=f"h{ht}", name=f"h{ht}")
                     for ht in range(HT)]
            m_all = sp.tile([P, GW], i32)

            # ---------------- stage 1: gating + routing ----------------
            with tc.tile_pool(name="gpsum", bufs=2, space="PSUM") as ppg, \
                 tc.tile_pool(name="gsb", bufs=3) as sg:
                dummy_ps = ppg.tile([1, 2], f32, tag="dummy", bufs=1)
                pe_touch(gws[0:1, 0:2])
                pe_touch(ident[0:1, 0:2])
                pe_touch(uexc[0:1, 0:2])
                pe_touch(onesc[0:1, 0:1])
                pe_touch(onesr[0:1, 0:2])
                # --- software-pipelined gating: stage A (logits + routing
                # weights on DVE) for chunk j overlaps stage B (slot-prefix
                # matmuls + scatters) for chunk j-1, so the PE FIFO never
                # stalls on the DVE routing chain.
                stA = {}

                def emit_logits(j):
                    xt = sg.tile([P, DT, GATE_CHUNK], f32, tag="xt", bufs=4)
                    eng = nc.sync if j % 2 == 0 else nc.scalar
                    eng.dma_start(out=xt[:], in_=xP4[:, j])
                    ps = ppg.tile([E, GATE_CHUNK], f32, tag="ps", space="PSUM")
                    for h0 in range(0, GATE_CHUNK, 512):
                        for k in range(DT):
                            nc.tensor.matmul(ps[:, h0:h0 + 512],
                                             lhsT=gws[:, k * E:(k + 1) * E],
                                             rhs=xt[:, k, h0:h0 + 512],
                                             start=(k == 0), stop=(k == DT - 1))
                    return ps

                def emit_transposes(j, ps):
                    sc_sb = sg.tile([E, GATE_CHUNK], f32, tag="sc", bufs=2)
                    nc.vector.tensor_copy(out=sc_sb[:], in_=ps[:])
                    pstb = ppg.tile([P, BPC * E], f32, tag="pst", space="PSUM")
                    for i in range(BPC):
                        nc.tensor.transpose(out=pstb[:, i * E:(i + 1) * E],
                                            in_=sc_sb[:, i * P:(i + 1) * P],
                                            identity=ident[0:E, 0:E])
                    return pstb

                def emit_routeA(j, pstb):
                    scores = sg.tile([P, BPC * E], f32, tag="scores")
                    nc.vector.tensor_copy(out=scores[:], in_=pstb[:])
                    mx = sg.tile([P, BPC * 8], f32, tag="mx")
                    for i in range(BPC):
                        nc.vector.max(out=mx[:, i * 8:(i + 1) * 8],
                                      in_=scores[:, i * E:(i + 1) * E])
                    sc3 = scores[:].rearrange("p (b e) -> p b e", e=E)
                    mx3 = mx[:].rearrange("p (b e) -> p b e", e=8)
                    m1j = mx3[:, :, 0]
                    m2j = mx3[:, :, 1]
                    sej = sc3[:, :, 0]               # own expert is column 0
                    dlt = sg.tile([P, BPC], f32, tag="dlt")
                    nc.vector.tensor_sub(out=dlt[:], in0=m2j, in1=m1j)
                    ed = sg.tile([P, BPC], f32, tag="ed")
                    nc.scalar.activation(out=ed[:], in_=dlt[:], func=Act.Exp)
                    den = sg.tile([P, BPC], f32, tag="den")
                    nc.vector.tensor_scalar_add(den[:], ed[:], 1.0)
                    wtop = sg.tile([P, BPC], f32, tag="wtop")
                    nc.vector.reciprocal(out=wtop[:], in_=den[:])
                    wsec = sg.tile([P, BPC], f32, tag="wsec")
                    nc.vector.tensor_scalar(out=wsec[:], in0=wtop[:], scalar1=-1.0,
                                            scalar2=1.0, op0=Alu.mult, op1=Alu.add)
                    istop = sg.tile([P, BPC], f32, tag="istop")
                    nc.vector.tensor_tensor(out=istop[:], in0=sej, in1=m1j, op=Alu.is_ge)
                    wdiff = sg.tile([P, BPC], f32, tag="wdiff")
                    nc.vector.tensor_sub(out=wdiff[:], in0=wtop[:], in1=wsec[:])
                    wE = sg.tile([P, BPC], f32, tag="wE")
                    nc.vector.tensor_tensor(out=wE[:], in0=istop[:], in1=wdiff[:], op=Alu.mult)
                    nc.vector.tensor_add(out=wE[:], in0=wE[:], in1=wsec[:])
                    maskj = sg.tile([P, BPC], f32, tag="maskj")
                    nc.vector.tensor_tensor(out=maskj[:], in0=sej, in1=m2j, op=Alu.is_ge)
                    stA[j] = (maskj, wE)

                def emit_slotMM1(j):
                    maskj, _ = stA[j]
                    pslot = ppg.tile([P, BPC], f32, tag="pslot", space="PSUM", bufs=2)
                    nc.tensor.matmul(pslot[:], lhsT=uexc[:], rhs=maskj[:], start=True, stop=False)
                    ptot = ppg.tile([1, BPC], f32, tag="dummy", space="PSUM", bufs=1)
                    nc.tensor.matmul(ptot[:], lhsT=onesc[:], rhs=maskj[:], start=True, stop=True)
                    return pslot, ptot

                def emit_scan(j, ptot):
                    b0 = j * BPC
                    tot = sg.tile([1, BPC], f32, tag="tot")
                    nc.vector.tensor_copy(out=tot[:], in_=ptot[:])
                    init = 0.0 if j == 0 else incl_all[:, b0 - 1:b0]
                    nc.vector.tensor_tensor_scan(incl_all[:, b0:b0 + BPC], tot[:], tot[:], init,
                                                 op0=Alu.add, op1=Alu.bypass)
                    excl = sg.tile([1, BPC], f32, tag="excl")
                    nc.vector.tensor_sub(out=excl[:], in0=incl_all[:, b0:b0 + BPC], in1=tot[:])
                    return excl

                def emit_slotB(j, pslot, excl):
                    b0 = j * BPC
                    maskj, wE = stA.pop(j)
                    nc.tensor.matmul(pslot[:], lhsT=onesr[:], rhs=excl[:], start=False, stop=True)
                    slot_f = sg.tile([P, BPC], f32, tag="slot_f")
                    nc.vector.tensor_copy(out=slot_f[:], in_=pslot[:])
                    off_f = sg.tile([P, BPC], f32, tag="off_f")
                    nc.vector.tensor_scalar(out=off_f[:], in0=maskj[:], scalar1=-1e6,
                                            scalar2=1e6, op0=Alu.mult, op1=Alu.add)
                    slot_oob = sg.tile([P, BPC], f32, tag="slot_oob")
                    nc.vector.tensor_add(out=slot_oob[:], in0=slot_f[:], in1=off_f[:])
                    # element offset into [P, GW]: off = s*34 - (s//128)*4350
                    # (i32 convert ROUNDS to nearest; center the fraction so
                    # round(s/128 - 0.496) == s//128 for all p in [0,128))
                    gq = sg.tile([P, BPC], f32, tag="gq")
                    nc.vector.tensor_scalar(out=gq[:], in0=slot_oob[:],
                                            scalar1=1.0 / P, scalar2=-0.49609375,
                                            op0=Alu.mult, op1=Alu.add)
                    gqi = sg.tile([P, BPC], i32, tag="gqi")
                    nc.vector.tensor_copy(out=gqi[:], in_=gq[:])
                    gqf = sg.tile([P, BPC], f32, tag="gqf")
                    nc.vector.tensor_copy(out=gqf[:], in_=gqi[:])
                    o1 = sg.tile([P, BPC], f32, tag="o1")
                    nc.vector.tensor_scalar(out=o1[:], in0=slot_oob[:],
                                            scalar1=float(GW), scalar2=None, op0=Alu.mult)
                    o2 = sg.tile([P, BPC], f32, tag="o2")
                    nc.vector.tensor_scalar(out=o2[:], in0=gqf[:],
                                            scalar1=float(P * GW - 2), scalar2=None,
                                            op0=Alu.mult)
                    o3 = sg.tile([P, BPC], f32, tag="o3")
                    nc.vector.tensor_sub(out=o3[:], in0=o1[:], in1=o2[:])
                    off_i = sg.tile([P, BPC], i32, tag="off_i")
                    nc.vector.tensor_copy(out=off_i[:], in_=o3[:])
                    iw = sg.tile([P, 2 * BPC], i32, tag="iw")
                    iw3 = iw[:].rearrange("p (b two) -> p b two", two=2)
                    nc.vector.tensor_copy(out=iw3[:, :, 0], in_=iota[:, b0:b0 + BPC])
                    nc.vector.tensor_copy(out=iw3[:, :, 1], in_=wE[:].bitcast(i32))
                    for i in range(BPC):
                        q = (b0 + i) % NIDW
                        nc.gpsimd.indirect_dma_start(
                            out=idw_el[q],
                            out_offset=bass.IndirectOffsetOnAxis(ap=off_i[:, i:i + 1], axis=0),
                            in_=iw[:, 2 * i:2 * i + 2], in_offset=None,
                            bounds_check=P * GW - 2, oob_is_err=False)

                prevB = None   # (j-1, pslot, ptot)
                for j in range(NJ):
                    ps = emit_logits(j)
                    if prevB is not None:
                        jm, pslot_m, ptot_m = prevB
                        excl_m = emit_scan(jm, ptot_m)
                    pstb = emit_transposes(j, ps)
                    if prevB is not None:
                        emit_slotB(jm, pslot_m, excl_m)
                    emit_routeA(j, pstb)
                    prevB = (j, *emit_slotMM1(j))
                jm, pslot_m, ptot_m = prevB
                excl_m = emit_scan(jm, ptot_m)
                emit_slotB(jm, pslot_m, excl_m)

                cnt_sb = sg.tile([1, 1], f32, tag="cnt")
                nc.vector.tensor_copy(out=cnt_sb[:], in_=incl_all[:, NB - 1:NB])
                nc.sync.dma_start(out=cnt_d[:], in_=cnt_sb[:])

            # ------- stage 2: idw merge + gather + PE transpose -------
            # ------- stage 3/4: FFN pass1 + pass2 (all bf16, one pass) ----
            with tc.tile_pool(name="f_ps", bufs=2, space="PSUM") as pp, \
                 tc.tile_pool(name="gat_sb", bufs=3) as sgt, \
                 tc.tile_pool(name="ffn_sb", bufs=3) as s1:
                dummy_ps = pp.tile([1, 2], f32, tag="dummy", bufs=1)

                q_sb = []
                for q in range(NIDW):
                    t = sgt.tile([P, GW], i32, tag=f"q{q}", bufs=1)
                    eng = nc.sync if q % 2 == 0 else nc.scalar
                    eng.dma_start(out=t[:], in_=idw_d[q][:])
                    q_sb.append(t)
                m01 = sgt.tile([P, GW], i32, tag="m01", bufs=1)
                nc.vector.tensor_add(out=m01[:], in0=q_sb[0][:], in1=q_sb[1][:])
                m23 = sgt.tile([P, GW], i32, tag="m23", bufs=1)
                nc.vector.tensor_add(out=m23[:], in0=q_sb[2][:], in1=q_sb[3][:])
                nc.vector.tensor_add(out=m_all[:], in0=m01[:], in1=m23[:])

                def emit_gathers(g0, g1):
                    # gather + PE-transpose slot tiles [g0, g1)
                    for g in range(g0, g1):
                        xg = sgt.tile([P, D], bf16, tag="xg")
                        nc.gpsimd.indirect_dma_start(
                            out=xg[:], out_offset=None, in_=xb_d[:],
                            in_offset=bass.IndirectOffsetOnAxis(ap=m_all[:, 2 * g:2 * g + 1], axis=0),
                            bounds_check=T - 1, oob_is_err=False)
                        for k in range(DT):
                            pst = pp.tile([P, P], bf16, tag="pstT", space="PSUM")
                            nc.tensor.transpose(out=pst[:], in_=xg[:, P * k:P * (k + 1)],
                                                identity=identb[:])
                            nc.vector.tensor_copy(out=xgT[k][:, g * P:(g + 1) * P],
                                                  in_=pst[:])

                # FFN pass 1: h = silu(x@w1T) * (x@w3T)
                # gathers are interleaved into ht=0's slice loop so pass1
                # matmuls start as soon as the first slice's tiles land
                g_done = 0
                prev_silu = None
                for ht in range(HT):
                    w1b = s1.tile([P, DT, P], bf16, tag="w1b")
                    nc.scalar.dma_start(out=w1b[:], in_=w1R4[:, ht])
                    w3b = s1.tile([P, DT, P], bf16, tag="w3b")
                    nc.scalar.dma_start(out=w3b[:], in_=w3R4[:, ht])
                    for (s0, sl) in SLICES:
                        if ht == 0:
                            g_need = (s0 + sl + P - 1) // P
                            emit_gathers(g_done, g_need)
                            g_done = max(g_done, g_need)
                            # absorb the gather-transpose DVE sems per slice
                            g_hi = (s0 + sl) // P - 1
                            for k in range(DT):
                                pe_touch(xgT[k][0:1, g_hi * P:g_hi * P + 2])
                        ph1 = pp.tile([P, 512], f32, tag="ph1", space="PSUM")
                        ph3 = pp.tile([P, 512], f32, tag="ph3", space="PSUM", bufs=1)
                        for k in range(DT):
                            nc.tensor.matmul(ph1[:, :sl], lhsT=w1b[:, k, :],
                                             rhs=xgT[k][:, s0:s0 + sl],
                                             start=(k == 0), stop=(k == DT - 1))
                        for k in range(DT):
                            nc.tensor.matmul(ph3[:, :sl], lhsT=w3b[:, k, :],
                                             rhs=xgT[k][:, s0:s0 + sl],
                                             start=(k == 0), stop=(k == DT - 1))
                        silu = s1.tile([P, 512], f32, tag="silu")
                        nc.scalar.activation(out=silu[:, :sl], in_=ph1[:, :sl], func=Act.Silu)
                        nc.vector.tensor_tensor(out=h_all[ht][:, s0:s0 + sl],
                                                in0=silu[:, :sl], in1=ph3[:, :sl],
                                                op=Alu.mult)
                        if prev_silu is not None:
                            pe_touch(prev_silu)
                        prev_silu = silu[0:1, 0:2]

                # FFN pass 2: yT = h @ w2T (feature-major, unscaled)
                for ht in range(HT):
                    pe_touch(h_all[ht][0:1, 0:2])
                for dt in range(DT):
                    w2b = s1.tile([P, HT, P], bf16, tag="w2b", bufs=2)
                    nc.scalar.dma_start(out=w2b[:], in_=w2R4[:, dt])
                    for (s0, sl) in SLICES:
                        py = pp.tile([P, 512], f32, tag="py", space="PSUM")
                        for j in range(HT):
                            nc.tensor.matmul(py[:, :sl], lhsT=w2b[:, j, :],
                                             rhs=h_all[j][:, s0:s0 + sl],
                                             start=(j == 0), stop=(j == HT - 1))
                        ysb = s1.tile([P, 512], f32, tag="ysb")
                        nc.vector.tensor_copy(out=ysb[:, :sl], in_=py[:, :sl])
                        nc.sync.dma_start(
                            out=yT_d[dt * P:(dt + 1) * P, s0:s0 + sl],
                            in_=ysb[:, :sl])

    nc.compile()
    return nc


def _marshal(x, gate_w, w1, w3, w2):
    xf = np.ascontiguousarray(x.reshape(T, D).astype(np.float32))
    xb = np.ascontiguousarray(xf.astype(bfloat16))
    xT = np.ascontiguousarray(xf.T)
    # chunk-contiguous gating stream: xP[p, j, k, t] = xT[k*128+p, j*512+t]
    xP = np.ascontiguousarray(
        xT.reshape(DT, P, NJ, GATE_CHUNK).transpose(1, 2, 0, 3)).reshape(P, -1)
    consts = {
        "uexc": np.triu(np.ones((P, P), np.float32), 1),
        "ones_col": np.ones((P, 1), np.float32),
        "ones_row": np.ones((1, P), np.float32),
        "iota": (np.arange(P)[:, None] + P * np.arange(NB)[None, :]).astype(np.int32),
        "ident": np.eye(P, dtype=np.float32),
        "identb": np.eye(P, dtype=np.float32).astype(bfloat16),
    }
    in_maps = []
    for e in range(E):
        perm = [e] + [i for i in range(E) if i != e]
        gwT = gate_w[perm].T.astype(np.float32)                      # [D, 8]
        gwP = np.ascontiguousarray(
            gwT.reshape(DT, P, E).transpose(1, 0, 2)).reshape(P, DT * E)
        # per-partition contiguous tile layout:
        # w1R[p, t, k, c] = w1T[k*128+p, t*128+c],  w1T = w1[e].T  [D, H]
        w1T = w1[e].astype(np.float32).T
        w3T = w3[e].astype(np.float32).T
        w2T = w2[e].astype(np.float32).T                             # [H, D]
        w1R = np.ascontiguousarray(
            w1T.reshape(DT, P, HT, P).transpose(1, 2, 0, 3)).reshape(P, HT * DT * P).astype(bfloat16)
        w3R = np.ascontiguousarray(
            w3T.reshape(DT, P, HT, P).transpose(1, 2, 0, 3)).reshape(P, HT * DT * P).astype(bfloat16)
        w2R = np.ascontiguousarray(
            w2T.reshape(HT, P, DT, P).transpose(1, 2, 0, 3)).reshape(P, DT * HT * P).astype(bfloat16)
        in_maps.append({
            "xb": xb, "xP": xP, "gwP": gwP,
            "w1R": w1R, "w3R": w3R, "w2R": w2R, **consts,
        })
    return in_maps


def _numpy_fallback(x, gate_w, w1, w3, w2):
    xf = x.reshape(T, D).astype(np.float64)
    logits = xf @ gate_w.astype(np.float64).T
    p = np.exp(logits - logits.max(1, keepdims=True))
    p /= p.sum(1, keepdims=True)
    idx = np.argsort(-p, axis=1, kind="stable")[:, :K]
    vals = np.take_along_axis(p, idx, 1)
    vals /= vals.sum(1, keepdims=True)
    y = np.zeros_like(xf)
    for e in range(E):
        m = (idx == e)
        wgt = (vals * m).sum(1)
        tsel = m.any(1)
        xe = xf[tsel]
        hm = xe @ w1[e].astype(np.float64).T
        hm = hm / (1 + np.exp(-hm)) * (xe @ w3[e].astype(np.float64).T)
        y[tsel] += wgt[tsel, None] * (hm @ w2[e].astype(np.float64).T)
    return y.astype(np.float32).reshape(x.shape)


def run_spmd(x, gate_w, w1, w3, w2, trace=False):
    """Compile (cached), run on 8 cores, return results."""
    from concourse.bass_utils import run_bass_kernel_spmd
    if "nc" not in _cache:
        _cache["nc"] = _build()
    in_maps = _marshal(x, gate_w, w1, w3, w2)
    res = run_bass_kernel_spmd(_cache["nc"], in_maps, list(range(E)), trace=trace)
    return res


def kernel(x, gate_w, w1, w3, w2):
    x = np.asarray(x)
    res = run_spmd(x, gate_w, w1, w3, w2)
    y = np.zeros((T, D), np.float32)
    for e in range(E):
        r = res.results[e]
        cnt = int(round(float(r["cnt"][0, 0])))
        if cnt > C:
            return _numpy_fallback(x, gate_w, w1, w3, w2)
        m = sum(r[f"idw{q}"].astype(np.int64) for q in range(NIDW)).astype(np.int32)
        idw = m.reshape(P, NG, 2).transpose(1, 0, 2).reshape(C, 2)
        ids = idw[:cnt, 0]
        w = idw[:cnt, 1].view(np.float32)
        rows = r["yT"][:, :cnt].T
        if len(np.unique(ids)) == cnt:
            y[ids] += w[:, None] * rows
        else:
            np.add.at(y, ids, w[:, None] * rows)
    return y.reshape(x.shape)


# revision 19
# speedup vs baseline: 1.4996x; 1.0218x over previous
"""Trainium2 Bass kernel for an 8-expert top-2 SwiGLU MoE (expert parallelism).

Structure (8 NeuronCores, one expert per core):
  - Stage 1 (gating): stream the full transposed token set (f32, host-
    marshaled chunk-contiguous so DMA descriptors are 8KB, split across the
    two HWDGE queues) in 16 chunks of 512 tokens, logits on the PE in f32
    (exact top-2 selection), gate matrix column-permuted per core so its OWN
    expert is column 0. Top-2 via vector MAX8 per 128-token block; routing
    weights + mask via batched DVE ops; compact slot ids via matmul
    prefix-sums; per-block indirect scatters write (token_id, weight_bits)
    round-robin into FOUR zero-initialized partition-major DRAM tensors
    (issue-bound instead of completion-chained on the gpsimd queue), using
    arithmetic element offsets off = slot*34 - (slot//128)*4350 so each
    tensor is [P, NG*2] and loads back in ONE contiguous DMA.
  - Stage 2 (gather): merge the 4 idw tensors (DVE int adds), indirect-
    gather token rows from a bf16 copy of x, PE-transpose (bf16) into
    feature-major xgT bf16.
  - Stage 3/4 (FFN, all bf16): single pass over all C=2176 slots in 5
    slices (4x512+128). Weights pre-marshaled bf16 with per-partition-
    contiguous tile layout, streamed exactly ONCE on the scalar-engine DMA
    queue. pass1: h=silu(x@w1T)*(x@w3T) -> h_all bf16 resident; pass2:
    yT = (h@w2T)^T in f32 to yT_d[D, C].
  - Host: merge idw quarters, y[ids] += w[:, None] * yT[:, :cnt].T per core.

Self-contained: hardcodes shapes for x[4,2048,1024], 8 experts, H=2816, top-2.
"""
import sys

sys.path.insert(0, "/opt/trn_rl_repo")

import numpy as np
from ml_dtypes import bfloat16

# ---------------------------------------------------------------- config
B, S, D = 4, 2048, 1024
T = B * S                # 8192 tokens
E = 8                    # experts == cores
H = 2816
K = 2
P = 128
NB = T // P              # 64 token blocks (token = 128*b + p)
C = 2176                 # per-expert slot capacity (observed max 2175)
NG = C // P              # 17 slot tiles
HT = H // P              # 22
DT = D // P              # 8
GATE_CHUNK = 512
NJ = T // GATE_CHUNK     # 16
BPC = GATE_CHUNK // P    # 4 blocks per gating chunk
SLICES = [(0, 512), (512, 512), (1024, 512), (1536, 384), (1920, 256)]
NIDW = 4                 # scatter fan-out (independent WAW chains)
GW = NG * 2              # per-partition idw row: NG pairs

_cache = {}


def _build():
    import concourse.bass as bass
    import concourse.bacc as bacc
    import concourse.mybir as mybir
    import concourse.tile as tile

    f32 = mybir.dt.float32
    bf16 = mybir.dt.bfloat16
    i32 = mybir.dt.int32
    Alu = mybir.AluOpType
    Act = mybir.ActivationFunctionType

    nc = bacc.Bacc("TRN2", target_bir_lowering=False, debug=False)

    xb_d = nc.dram_tensor("xb", [T, D], bf16, kind="ExternalInput")
    xP_d = nc.dram_tensor("xP", [P, NJ * DT * GATE_CHUNK], f32, kind="ExternalInput")
    gwP_d = nc.dram_tensor("gwP", [P, DT * E], f32, kind="ExternalInput")
    w1R_d = nc.dram_tensor("w1R", [P, HT * DT * P], bf16, kind="ExternalInput")
    w3R_d = nc.dram_tensor("w3R", [P, HT * DT * P], bf16, kind="ExternalInput")
    w2R_d = nc.dram_tensor("w2R", [P, DT * HT * P], bf16, kind="ExternalInput")
    uexc_d = nc.dram_tensor("uexc", [P, P], f32, kind="ExternalInput")
    onesc_d = nc.dram_tensor("ones_col", [P, 1], f32, kind="ExternalInput")
    onesr_d = nc.dram_tensor("ones_row", [1, P], f32, kind="ExternalInput")
    iota_d = nc.dram_tensor("iota", [P, NB], i32, kind="ExternalInput")
    ident_d = nc.dram_tensor("ident", [P, P], f32, kind="ExternalInput")
    identb_d = nc.dram_tensor("identb", [P, P], bf16, kind="ExternalInput")

    idw_d = [nc.dram_tensor(f"idw{q}", [P, GW], i32, kind="ExternalOutput")
             for q in range(NIDW)]
    cnt_d = nc.dram_tensor("cnt", [1, 1], f32, kind="ExternalOutput")
    yT_d = nc.dram_tensor("yT", [D, C], f32, kind="ExternalOutput")

    xP4 = xP_d[:].rearrange("p (j k t) -> p j k t", j=NJ, k=DT)
    w1R4 = w1R_d[:].rearrange("p (t k c) -> p t k c", t=HT, k=DT)
    w3R4 = w3R_d[:].rearrange("p (t k c) -> p t k c", t=HT, k=DT)
    w2R4 = w2R_d[:].rearrange("p (d j c) -> p d j c", d=DT, j=HT)
    idw_el = [t[:].rearrange("p x -> (p x) ()") for t in idw_d]

    with tile.TileContext(nc) as tc:
        with tc.tile_pool(name="persist", bufs=1) as sp:
            # --- constants ---
            uexc = sp.tile([P, P], f32)
            nc.sync.dma_start(out=uexc[:], in_=uexc_d[:])
            onesc = sp.tile([P, 1], f32)
            nc.sync.dma_start(out=onesc[:], in_=onesc_d[:])
            onesr = sp.tile([1, P], f32)
            nc.sync.dma_start(out=onesr[:], in_=onesr_d[:])
            iota = sp.tile([P, NB], i32)
            nc.sync.dma_start(out=iota[:], in_=iota_d[:])
            ident = sp.tile([P, P], f32)
            nc.sync.dma_start(out=ident[:], in_=ident_d[:])
            identb = sp.tile([P, P], bf16)
            nc.sync.dma_start(out=identb[:], in_=identb_d[:])
            gws = sp.tile([P, DT * E], f32)
            nc.sync.dma_start(out=gws[:], in_=gwP_d[:])

            # zero-init the idw scatter targets (merge-by-add needs 0s)
            zeros_sb = sp.tile([P, GW], i32)
            nc.vector.tensor_scalar(out=zeros_sb[:], in0=iota[:, 0:GW],
                                    scalar1=0, scalar2=None, op0=Alu.mult)
            for q in range(NIDW):
                eng = nc.sync if q % 2 == 0 else nc.scalar
                eng.dma_start(out=idw_d[q][:], in_=zeros_sb[:])

            # PE wait-absorber: matmul codegen allows a single sync wait, so
            # before any matmul that would need 2+ waits we make the PE observe
            # the extra semaphores through a tiny dummy matmul.
            dummy_ps = None

            def pe_touch(ap):
                n = ap.shape[-1]
                nc.tensor.matmul(dummy_ps[0:1, 0:n], lhsT=ap[:, 0:1], rhs=ap,
                                 start=True, stop=True, skip_group_check=True)

            incl_all = sp.tile([1, NB], f32)

            # FFN persistent activations (bf16)
            xgT = [sp.tile([P, C], bf16, tag=f"xgT{k}", name=f"xgT{k}")
                   for k in range(DT)]
            h_all = [sp.tile([P, C], bf16, tag=f"h{ht}", name=f"h{ht}")
                     for ht in range(HT)]
            m_all = sp.tile([P, GW], i32)

            # ---------------- stage 1: gating + routing ----------------
            with tc.tile_pool(name="gpsum", bufs=2, space="PSUM") as ppg, \
                 tc.tile_pool(name="gsb", bufs=3) as sg:
                dummy_ps = ppg.tile([1, 2], f32, tag="dummy", bufs=1)
                pe_touch(gws[0:1, 0:2])
                pe_touch(ident[0:1, 0:2])
                pe_touch(uexc[0:1, 0:2])
                pe_touch(onesc[0:1, 0:1])
                pe_touch(onesr[0:1, 0:2])
                # --- software-pipelined gating: stage A (logits + routing
                # weights on DVE) for chunk j overlaps stage B (slot-prefix
                # matmuls + scatters) for chunk j-1, so the PE FIFO never
                # stalls on the DVE routing chain.
                stA = {}

                def emit_logits(j):
                    xt = sg.tile([P, DT, GATE_CHUNK], f32, tag="xt", bufs=4)
                    eng = nc.sync if j % 2 == 0 else nc.scalar
                    eng.dma_start(out=xt[:], in_=xP4[:, j])
                    ps = ppg.tile([E, GATE_CHUNK], f32, tag="ps", space="PSUM")
                    for h0 in range(0, GATE_CHUNK, 512):
                        for k in range(DT):
                            nc.tensor.matmul(ps[:, h0:h0 + 512],
                                             lhsT=gws[:, k * E:(k + 1) * E],
                                             rhs=xt[:, k, h0:h0 + 512],
                                             start=(k == 0), stop=(k == DT - 1))
                    return ps

                def emit_transposes(j, ps):
                    sc_sb = sg.tile([E, GATE_CHUNK], f32, tag="sc", bufs=2)
                    nc.vector.tensor_copy(out=sc_sb[:], in_=ps[:])
                    pstb = ppg.tile([P, BPC * E], f32, tag="pst", space="PSUM")
                    for i in range(BPC):
                        nc.tensor.transpose(out=pstb[:, i * E:(i + 1) * E],
                                            in_=sc_sb[:, i * P:(i + 1) * P],
                                            identity=ident[0:E, 0:E])
                    return pstb

                def emit_routeA(j, pstb):
                    scores = sg.tile([P, BPC * E], f32, tag="scores")
                    nc.vector.tensor_copy(out=scores[:], in_=pstb[:])
                    mx = sg.tile([P, BPC * 8], f32, tag="mx")
                    for i in range(BPC):
                        nc.vector.max(out=mx[:, i * 8:(i + 1) * 8],
                                      in_=scores[:, i * E:(i + 1) * E])
                    sc3 = scores[:].rearrange("p (b e) -> p b e", e=E)
                    mx3 = mx[:].rearrange("p (b e) -> p b e", e=8)
                    m1j = mx3[:, :, 0]
                    m2j = mx3[:, :, 1]
                    sej = sc3[:, :, 0]               # own expert is column 0
                    dlt = sg.tile([P, BPC], f32, tag="dlt")
                    nc.vector.tensor_sub(out=dlt[:], in0=m2j, in1=m1j)
                    ed = sg.tile([P, BPC], f32, tag="ed")
                    nc.scalar.activation(out=ed[:], in_=dlt[:], func=Act.Exp)
                    den = sg.tile([P, BPC], f32, tag="den")
                    nc.vector.tensor_scalar_add(den[:], ed[:], 1.0)
                    wtop = sg.tile([P, BPC], f32, tag="wtop")
                    nc.vector.reciprocal(out=wtop[:], in_=den[:])
                    wsec = sg.tile([P, BPC], f32, tag="wsec")
                    nc.vector.tensor_scalar(out=wsec[:], in0=wtop[:], scalar1=-1.0,
                                            scalar2=1.0, op0=Alu.mult, op1=Alu.add)
                    istop = sg.tile([P, BPC], f32, tag="istop")
                    nc.vector.tensor_tensor(out=istop[:], in0=sej, in1=m1j, op=Alu.is_ge)
                    wdiff = sg.tile([P, BPC], f32, tag="wdiff")
                    nc.vector.tensor_sub(out=wdiff[:], in0=wtop[:], in1=wsec[:])
                    wE = sg.tile([P, BPC], f32, tag="wE")
                    nc.vector.tensor_tensor(out=wE[:], in0=istop[:], in1=wdiff[:], op=Alu.mult)
                    nc.vector.tensor_add(out=wE[:], in0=wE[:], in1=wsec[:])
                    maskj = sg.tile([P, BPC], f32, tag="maskj")
                    nc.vector.tensor_tensor(out=maskj[:], in0=sej, in1=m2j, op=Alu.is_ge)
                    stA[j] = (maskj, wE)

                def emit_slotMM1(j):
                    maskj, _ = stA[j]
                    pslot = ppg.tile([P, BPC], f32, tag="pslot", space="PSUM", bufs=2)
                    nc.tensor.matmul(pslot[:], lhsT=uexc[:], rhs=maskj[:], start=True, stop=False)
                    ptot = ppg.tile([1, BPC], f32, tag="dummy", space="PSUM", bufs=1)
                    nc.tensor.matmul(ptot[:], lhsT=onesc[:], rhs=maskj[:], start=True, stop=True)
                    return pslot, ptot

                def emit_scan(j, ptot):
                    b0 = j * BPC
                    tot = sg.tile([1, BPC], f32, tag="tot")
                    nc.vector.tensor_copy(out=tot[:], in_=ptot[:])
                    init = 0.0 if j == 0 else incl_all[:, b0 - 1:b0]
                    nc.vector.tensor_tensor_scan(incl_all[:, b0:b0 + BPC], tot[:], tot[:], init,
                                                 op0=Alu.add, op1=Alu.bypass)
                    excl = sg.tile([1, BPC], f32, tag="excl")
                    nc.vector.tensor_sub(out=excl[:], in0=incl_all[:, b0:b0 + BPC], in1=tot[:])
                    return excl

                def emit_slotB(j, pslot, excl):
                    b0 = j * BPC
                    maskj, wE = stA.pop(j)
                    nc.tensor.matmul(pslot[:], lhsT=onesr[:], rhs=excl[:], start=False, stop=True)
                    slot_f = sg.tile([P, BPC], f32, tag="slot_f")
                    nc.vector.tensor_copy(out=slot_f[:], in_=pslot[:])
                    off_f = sg.tile([P, BPC], f32, tag="off_f")
                    nc.vector.tensor_scalar(out=off_f[:], in0=maskj[:], scalar1=-1e6,
                                            scalar2=1e6, op0=Alu.mult, op1=Alu.add)
                    slot_oob = sg.tile([P, BPC], f32, tag="slot_oob")
                    nc.vector.tensor_add(out=slot_oob[:], in0=slot_f[:], in1=off_f[:])
                    # element offset into [P, GW]: off = s*34 - (s//128)*4350
                    # (i32 convert ROUNDS to nearest; center the fraction so
                    # round(s/128 - 0.496) == s//128 for all p in [0,128))
                    gq = sg.tile([P, BPC], f32, tag="gq")
                    nc.vector.tensor_scalar(out=gq[:], in0=slot_oob[:],
                                            scalar1=1.0 / P, scalar2=-0.49609375,
                                            op0=Alu.mult, op1=Alu.add)
                    gqi = sg.tile([P, BPC], i32, tag="gqi")
                    nc.vector.tensor_copy(out=gqi[:], in_=gq[:])
                    gqf = sg.tile([P, BPC], f32, tag="gqf")
                    nc.vector.tensor_copy(out=gqf[:], in_=gqi[:])
                    o1 = sg.tile([P, BPC], f32, tag="o1")
                    nc.vector.tensor_scalar(out=o1[:], in0=slot_oob[:],
                                            scalar1=float(GW), scalar2=None, op0=Alu.mult)
                    o2 = sg.tile([P, BPC], f32, tag="o2")
                    nc.vector.tensor_scalar(out=o2[:], in0=gqf[:],
                                            scalar1=float(P * GW - 2), scalar2=None,
                                            op0=Alu.mult)
                    o3 = sg.tile([P, BPC], f32, tag="o3")
                    nc.vector.tensor_sub(out=o3[:], in0=o1[:], in1=o2[:])
                    off_i = sg.tile([P, BPC], i32, tag="off_i")
                    nc.vector.tensor_copy(out=off_i[:], in_=o3[:])
                    iw = sg.tile([P, 2 * BPC], i32, tag="iw")
                    iw3 = iw[:].rearrange("p (b two) -> p b two", two=2)
                    nc.vector.tensor_copy(out=iw3[:, :, 0], in_=iota[:, b0:b0 + BPC])
                    nc.vector.tensor_copy(out=iw3[:, :, 1], in_=wE[:].bitcast(i32))
                    for i in range(BPC):
                        q = (b0 + i) % NIDW
                        nc.gpsimd.indirect_dma_start(
                            out=idw_el[q],
                            out_offset=bass.IndirectOffsetOnAxis(ap=off_i[:, i:i + 1], axis=0),
                            in_=iw[:, 2 * i:2 * i + 2], in_offset=None,
                            bounds_check=P * GW - 2, oob_is_err=False)

                prevB = None   # (j-1, pslot, ptot)
                for j in range(NJ):
                    ps = emit_logits(j)
                    if prevB is not None:
                        jm, pslot_m, ptot_m = prevB
                        excl_m = emit_scan(jm, ptot_m)
                    pstb = emit_transposes(j, ps)
                    if prevB is not None:
                        emit_slotB(jm, pslot_m, excl_m)
                    emit_routeA(j, pstb)
                    prevB = (j, *emit_slotMM1(j))
                jm, pslot_m, ptot_m = prevB
                excl_m = emit_scan(jm, ptot_m)
                emit_slotB(jm, pslot_m, excl_m)

                cnt_sb = sg.tile([1, 1], f32, tag="cnt")
                nc.vector.tensor_copy(out=cnt_sb[:], in_=incl_all[:, NB - 1:NB])
                nc.sync.dma_start(out=cnt_d[:], in_=cnt_sb[:])

            # ------- stage 2: idw merge + gather + PE transpose -------
            # ------- stage 3/4: FFN pass1 + pass2 (all bf16, one pass) ----
            with tc.tile_pool(name="f_ps", bufs=2, space="PSUM") as pp, \
                 tc.tile_pool(name="gat_sb", bufs=3) as sgt, \
                 tc.tile_pool(name="ffn_sb", bufs=3) as s1:
                dummy_ps = pp.tile([1, 2], f32, tag="dummy", bufs=1)

                q_sb = []
                for q in range(NIDW):
                    t = sgt.tile([P, GW], i32, tag=f"q{q}", bufs=1)
                    eng = nc.sync if q % 2 == 0 else nc.scalar
                    eng.dma_start(out=t[:], in_=idw_d[q][:])
                    q_sb.append(t)
                m01 = sgt.tile([P, GW], i32, tag="m01", bufs=1)
                nc.vector.tensor_add(out=m01[:], in0=q_sb[0][:], in1=q_sb[1][:])
                m23 = sgt.tile([P, GW], i32, tag="m23", bufs=1)
                nc.vector.tensor_add(out=m23[:], in0=q_sb[2][:], in1=q_sb[3][:])
                nc.vector.tensor_add(out=m_all[:], in0=m01[:], in1=m23[:])

                def emit_gathers(g0, g1):
                    # gather + PE-transpose slot tiles [g0, g1)
                    for g in range(g0, g1):
                        xg = sgt.tile([P, D], bf16, tag="xg")
                        nc.gpsimd.indirect_dma_start(
                            out=xg[:], out_offset=None, in_=xb_d[:],
                            in_offset=bass.IndirectOffsetOnAxis(ap=m_all[:, 2 * g:2 * g + 1], axis=0),
                            bounds_check=T - 1, oob_is_err=False)
                        for k in range(DT):
                            pst = pp.tile([P, P], bf16, tag="pstT", space="PSUM")
                            nc.tensor.transpose(out=pst[:], in_=xg[:, P * k:P * (k + 1)],
                                                identity=identb[:])
                            nc.vector.tensor_copy(out=xgT[k][:, g * P:(g + 1) * P],
                                                  in_=pst[:])

                # FFN pass 1: h = silu(x@w1T) * (x@w3T)
                # gathers are interleaved into ht=0's slice loop so pass1
                # matmuls start as soon as the first slice's tiles land
                g_done = 0
                prev_silu = None
                for ht in range(HT):
                    w1b = s1.tile([P, DT, P], bf16, tag="w1b")
                    nc.scalar.dma_start(out=w1b[:], in_=w1R4[:, ht])
                    w3b = s1.tile([P, DT, P], bf16, tag="w3b")
                    nc.scalar.dma_start(out=w3b[:], in_=w3R4[:, ht])
                    for (s0, sl) in SLICES:
                        if ht == 0:
                            g_need = (s0 + sl + P - 1) // P
                            emit_gathers(g_done, g_need)
                            g_done = max(g_done, g_need)
                            # absorb the gather-transpose DVE sems per slice
                            g_hi = (s0 + sl) // P - 1
                            for k in range(DT):
                                pe_touch(xgT[k][0:1, g_hi * P:g_hi * P + 2])
                        ph1 = pp.tile([P, 512], f32, tag="ph1", space="PSUM")
                        ph3 = pp.tile([P, 512], f32, tag="ph3", space="PSUM", bufs=1)
                        for k in range(DT):
                            nc.tensor.matmul(ph1[:, :sl], lhsT=w1b[:, k, :],
                                             rhs=xgT[k][:, s0:s0 + sl],
                                             start=(k == 0), stop=(k == DT - 1))
                        for k in range(DT):
                            nc.tensor.matmul(ph3[:, :sl], lhsT=w3b[:, k, :],
                                             rhs=xgT[k][:, s0:s0 + sl],
                                             start=(k == 0), stop=(k == DT - 1))
                        silu = s1.tile([P, 512], f32, tag="silu")
                        nc.scalar.activation(out=silu[:, :sl], in_=ph1[:, :sl], func=Act.Silu)
                        nc.vector.tensor_tensor(out=h_all[ht][:, s0:s0 + sl],
                                                in0=silu[:, :sl], in1=ph3[:, :sl],
                                                op=Alu.mult)
                        if prev_silu is not None and (s0 == 0):
                            pe_touch(prev_silu)
                        prev_silu = silu[0:1, 0:2]

                # FFN pass 2: yT = h @ w2T (feature-major, unscaled)
                for ht in range(HT):
                    pe_touch(h_all[ht][0:1, 0:2])
                for dt in range(DT):
                    w2b = s1.tile([P, HT, P], bf16, tag="w2b", bufs=2)
                    nc.sync.dma_start(out=w2b[:], in_=w2R4[:, dt])
                    for (s0, sl) in SLICES:
                        py = pp.tile([P, 512], f32, tag="py", space="PSUM")
                        for j in range(HT):
                            nc.tensor.matmul(py[:, :sl], lhsT=w2b[:, j, :],
                                             rhs=h_all[j][:, s0:s0 + sl],
                                             start=(j == 0), stop=(j == HT - 1))
                        ysb = s1.tile([P, 512], f32, tag="ysb")
                        nc.vector.tensor_copy(out=ysb[:, :sl], in_=py[:, :sl])
                        nc.sync.dma_start(
                            out=yT_d[dt * P:(dt + 1) * P, s0:s0 + sl],
                            in_=ysb[:, :sl])

    nc.compile()
    return nc


def _marshal(x, gate_w, w1, w3, w2):
    xf = np.ascontiguousarray(x.reshape(T, D).astype(np.float32))
    xb = np.ascontiguousarray(xf.astype(bfloat16))
    xT = np.ascontiguousarray(xf.T)
    # chunk-contiguous gating stream: xP[p, j, k, t] = xT[k*128+p, j*512+t]
    xP = np.ascontiguousarray(
        xT.reshape(DT, P, NJ, GATE_CHUNK).transpose(1, 2, 0, 3)).reshape(P, -1)
    consts = {
        "uexc": np.triu(np.ones((P, P), np.float32), 1),
        "ones_col": np.ones((P, 1), np.float32),
        "ones_row": np.ones((1, P), np.float32),
        "iota": (np.arange(P)[:, None] + P * np.arange(NB)[None, :]).astype(np.int32),
        "ident": np.eye(P, dtype=np.float32),
        "identb": np.eye(P, dtype=np.float32).astype(bfloat16),
    }
    in_maps = []
    for e in range(E):
        perm = [e] + [i for i in range(E) if i != e]
        gwT = gate_w[perm].T.astype(np.float32)                      # [D, 8]
        gwP = np.ascontiguousarray(
            gwT.reshape(DT, P, E).transpose(1, 0, 2)).reshape(P, DT * E)
        # per-partition contiguous tile layout:
        # w1R[p, t, k, c] = w1T[k*128+p, t*128+c],  w1T = w1[e].T  [D, H]
        w1T = w1[e].astype(np.float32).T
        w3T = w3[e].astype(np.float32).T
        w2T = w2[e].astype(np.float32).T                             # [H, D]
        w1R = np.ascontiguousarray(
            w1T.reshape(DT, P, HT, P).transpose(1, 2, 0, 3)).reshape(P, HT * DT * P).astype(bfloat16)
        w3R = np.ascontiguousarray(
            w3T.reshape(DT, P, HT, P).transpose(1, 2, 0, 3)).reshape(P, HT * DT * P).astype(bfloat16)
        w2R = np.ascontiguousarray(
            w2T.reshape(HT, P, DT, P).transpose(1, 2, 0, 3)).reshape(P, DT * HT * P).astype(bfloat16)
        in_maps.append({
            "xb": xb, "xP": xP, "gwP": gwP,
            "w1R": w1R, "w3R": w3R, "w2R": w2R, **consts,
        })
    return in_maps


def _numpy_fallback(x, gate_w, w1, w3, w2):
    xf = x.reshape(T, D).astype(np.float64)
    logits = xf @ gate_w.astype(np.float64).T
    p = np.exp(logits - logits.max(1, keepdims=True))
    p /= p.sum(1, keepdims=True)
    idx = np.argsort(-p, axis=1, kind="stable")[:, :K]
    vals = np.take_along_axis(p, idx, 1)
    vals /= vals.sum(1, keepdims=True)
    y = np.zeros_like(xf)
    for e in range(E):
        m = (idx == e)
        wgt = (vals * m).sum(1)
        tsel = m.any(1)
        xe = xf[tsel]
        hm = xe @ w1[e].astype(np.float64).T
        hm = hm / (1 + np.exp(-hm)) * (xe @ w3[e].astype(np.float64).T)
        y[tsel] += wgt[tsel, None] * (hm @ w2[e].astype(np.float64).T)
    return y.astype(np.float32).reshape(x.shape)


def run_spmd(x, gate_w, w1, w3, w2, trace=False):
    """Compile (cached), run on 8 cores, return results."""
    from concourse.bass_utils import run_bass_kernel_spmd
    if "nc" not in _cache:
        _cache["nc"] = _build()
    in_maps = _marshal(x, gate_w, w1, w3, w2)
    res = run_bass_kernel_spmd(_cache["nc"], in_maps, list(range(E)), trace=trace)
    return res


def kernel(x, gate_w, w1, w3, w2):
    x = np.asarray(x)
    res = run_spmd(x, gate_w, w1, w3, w2)
    y = np.zeros((T, D), np.float32)
    for e in range(E):
        r = res.results[e]
        cnt = int(round(float(r["cnt"][0, 0])))
        if cnt > C:
            return _numpy_fallback(x, gate_w, w1, w3, w2)
        m = sum(r[f"idw{q}"].astype(np.int64) for q in range(NIDW)).astype(np.int32)
        idw = m.reshape(P, NG, 2).transpose(1, 0, 2).reshape(C, 2)
        ids = idw[:cnt, 0]
        w = idw[:cnt, 1].view(np.float32)
        rows = r["yT"][:, :cnt].T
        if len(np.unique(ids)) == cnt:
            y[ids] += w[:, None] * rows
        else:
            np.add.at(y, ids, w[:, None] * rows)
    return y.reshape(x.shape)
